# revision 22
# baseline (speedup 1.0000x reference)
"""DRL4TSP pointer-network decode on 8 Trainium2 NeuronCores.

Data-parallel over batch (16 items/core, 2 software-pipelined groups of 8).
All parameters replicated; the 64-step greedy decode runs fully on-device.

Structure (per core, fp32 throughout):
  - Hoisted loop-invariants (computed on device by PE):
      U    = W_as@static_h + W_ad@dynamic_h      [H,(b,s)]
      V    = P_s@static_h                        [H,(b,s)]
      PST  = (P_c@static_h) transposed per item  [S,(b,H)]
      GtT  = ((gru_wih@decoder_w)@static + bias) transposed per
             (gate,item)                         [S,(gate,b,H)]
  - Per decode step, the serial chain is split into 4 phases
    (gru / attn-front / attn-back / argmax-tail) and the two groups are
    emitted software-pipelined so every engine's in-order stream always
    has ready work:
      argmax: pointer logits [S,(b)] psum -> gpsimd partition_all_reduce
      (max) -> DVE is_equal one-hot -> next gi via one-hot matmuls
      against GtT (bit-exact gather); ptr index via one-hot @ iota.
      logp = max - ln(sum exp(l)) banked per step, one Ln at the end.
"""

import numpy as np


def _ensure_path():
    import sys

    try:
        import concourse.bass  # noqa: F401
        return
    except ImportError:
        pass
    for p in ("/opt/trn_rl_repo", "/root/.axon_site/_ro/trn_rl_repo"):
        if p not in sys.path:
            sys.path.insert(0, p)
    import concourse.bass  # noqa: F401


B, S, H = 128, 64, 128
NCORES = 8
BL = B // NCORES          # 16 items per core
NG = 2                    # groups per core
GB = BL // NG             # 8 items per group
W = GB * S                # 512 free width per group
F32 = "float32"

# constant packs, split by row count to minimize DMA bytes:
#   pack "a": 128-row tensors; "b": 2-row; "c": 1-row
_CP_PACKS = {
    "a": [("wasT", H), ("wadT", H), ("wpsT", H), ("wpcT", H), ("wrT", H),
          ("whhT", 3 * H), ("whhn05T", H), ("ones64", H), ("vecs", 8),
          ("ident", H)],
    "b": [("st", BL * S), ("dy", BL * S), ("swT", H), ("dwT", H),
          ("w2T", 3 * H)],
    "c": [("biasrow", 9 * H), ("ones_row", W)],
}
CPACK_ROWS = {"a": H, "b": 2, "c": 1}
CPACK_LAYOUT = {}
CPACK_COLS = {}
for _p, _lst in _CP_PACKS.items():
    _c = 0
    for _n, _w in _lst:
        CPACK_LAYOUT[_n] = (_p, _c, _w)
        _c += _w
    CPACK_COLS[_p] = _c

_CACHE: dict = {}


def _build_program(n_steps: int = S):
    _ensure_path()
    import concourse.bass as bass
    import concourse.bacc as bacc
    import concourse.mybir as mybir
    import concourse.bass_isa as bass_isa
    from concourse.tile import TileContext

    dt = mybir.dt
    AF = mybir.ActivationFunctionType
    ALU = mybir.AluOpType

    nc = bacc.Bacc("TRN2", target_bir_lowering=False, debug=False,
                   enable_asserts=False, num_devices=NCORES)

    # ---------------- DRAM I/O ----------------
    cpk = {p: nc.dram_tensor(f"cpack_{p}", [CPACK_ROWS[p], CPACK_COLS[p]],
                             dt.float32, kind="ExternalInput").ap()
           for p in CPACK_ROWS}
    out_idx = nc.dram_tensor("out_idx", [BL, S], dt.int32,
                             kind="ExternalOutput").ap()
    out_logp = nc.dram_tensor("out_logp", [BL, S], dt.float32,
                              kind="ExternalOutput").ap()

    with TileContext(nc) as tc:
        import contextlib

        ctx = contextlib.ExitStack()
        with ctx:
            cpool = ctx.enter_context(tc.tile_pool(name="consts", bufs=1))
            spool = ctx.enter_context(tc.tile_pool(name="work", bufs=3))
            gpool = ctx.enter_context(tc.tile_pool(name="gru", bufs=3))
            ppool_big = ctx.enter_context(
                tc.tile_pool(name="psbig", bufs=3, space="PSUM"))
            ppool_fix = ctx.enter_context(
                tc.tile_pool(name="psfix", bufs=1, space="PSUM"))

            # ---- load constants (3 DMAs, one per pack) ----
            cp_t = {}
            for p in CPACK_ROWS:
                cp_t[p] = cpool.tile([CPACK_ROWS[p], CPACK_COLS[p]],
                                     dt.float32, tag=f"cp{p}", name=f"cp{p}")
                nc.sync.dma_start(cp_t[p][:], cpk[p])

            def cslice(name, nrows):
                p, c0, w_ = CPACK_LAYOUT[name]
                return cp_t[p][0:nrows, c0:c0 + w_]

            st_s = cslice("st", 2)
            dy_s = cslice("dy", 2)
            swT_s = cslice("swT", 2)
            dwT_s = cslice("dwT", 2)
            w2T_s = cslice("w2T", 2)
            wasT_s = cslice("wasT", H)
            wadT_s = cslice("wadT", H)
            wpsT_s = cslice("wpsT", H)
            wpcT_s = cslice("wpcT", H)
            wrT_s = cslice("wrT", H)
            whhT_s = cslice("whhT", H)
            whhn05T_s = cslice("whhn05T", H)
            ones64_s = cslice("ones64", S)
            vecs_s = cslice("vecs", H)
            biasrow_s = cslice("biasrow", 1)
            ones_s = cslice("ones_row", 1)
            ident_s = cslice("ident", H)

            # biasrow columns: [0:H]=static_b [H:2H]=dynamic_b
            #   [2H:5H]=Gtab gate biases (r,z incl bhh; n = gbias_n)
            #   [5H:8H]=gi0 rows (r,z incl bhh fold; n plain)
            #   [8H:9H]=0.5*bhh_n
            # vecs columns: 4=attn_v 5=ptr_v 6=iota64(rows 0:64)

            # ---- persistent state ----
            h_s = cpool.tile([H, BL], dt.float32, tag="h", name="h")
            nc.vector.memset(h_s[:], 0.0)
            h2_s = cpool.tile([H, BL], dt.float32, tag="h2", name="h2")
            nc.vector.memset(h2_s[:], 0.0)

            U_s = [cpool.tile([H, W], dt.float32, tag=f"U{g}", name=f"U{g}")
                   for g in range(NG)]
            V_s = [cpool.tile([H, W], dt.float32, tag=f"V{g}", name=f"V{g}")
                   for g in range(NG)]
            PST_s = [cpool.tile([S, GB * H], dt.float32, tag=f"PST{g}",
                                name=f"PST{g}") for g in range(NG)]
            GtT_s = [cpool.tile([S, 3 * GB * H], dt.float32, tag=f"GtT{g}",
                                name=f"GtT{g}") for g in range(NG)]
            Zbuf_s = [cpool.tile([1, S * GB], dt.float32, tag=f"Zb{g}",
                                 name=f"Zb{g}") for g in range(NG)]
            mxbuf_s = [cpool.tile([1, S * GB], dt.float32, tag=f"mxb{g}",
                                  name=f"mxb{g}") for g in range(NG)]
            oi_s = [cpool.tile([GB, S], dt.int32, tag=f"oi{g}", name=f"oi{g}")
                    for g in range(NG)]

            # persistent per-group psum scratch (one full bank each):
            #   pw [H,0:8] | qt [0:64,8:16] | w2p [H,16:24] | z [H,24:32]
            #   lt [0:64,32:40] | zr [0:1,40:48] | ic [0:8,48:49]
            fix = [ppool_fix.tile([H, 128], dt.float32, tag=f"fix{g}",
                                  name=f"fix{g}") for g in range(NG)]
            ghq_t = ppool_fix.tile([H, 128], dt.float32, tag="ghq",
                                   name="ghq")
            ghq_reg = [[ghq_t[:, (2 * g + e) * 32:(2 * g + e + 1) * 32]
                        for e in range(2)] for g in range(NG)]
            zbank = [ppool_fix.tile([H, 512], dt.float32, tag=f"zbk{g}",
                                    name=f"zbk{g}") for g in range(NG)]
            pw_r = [fx[:, 0:GB] for fx in fix]
            qt_r = [fx[0:S, GB:2 * GB] for fx in fix]
            w2p_r = [fx[:, 2 * GB:3 * GB] for fx in fix]
            z_r = [fx[:, 3 * GB:4 * GB] for fx in fix]
            lt_r = [fx[0:S, 4 * GB:5 * GB] for fx in fix]
            zr_r = [zb[0:1, :] for zb in zbank]
            ic_r = [zb[64:64 + GB, 0:S] for zb in zbank]

            # ---------------- precompute ----------------
            # PE p-state warm-up: ~10us of back-to-back dummy matmuls so the
            # ramp hits full clock before the real prologue matmuls.
            wrm = cpool.tile([H, 256], dt.float32, tag="wrm", name="wrm")
            nc.vector.memset(wrm[:], 0.0)
            pwu = ppool_big.tile([H, W], dt.float32, tag="pc", name="pc")
            for _ in range(14):
                nc.tensor.matmul(pwu[:, 0:256], wrm[:, 0:H], wrm[:],
                                 start=True, stop=True,
                                 skip_group_check=True)

            def colrange(g):
                return slice(g * W, (g + 1) * W)

            sh_s, dh_s = [], []
            for g in range(NG):
                cs = colrange(g)
                ps = ppool_big.tile([H, W], dt.float32, tag="pc", name="pc")
                nc.tensor.matmul(ps[:], swT_s[:], st_s[:, cs], start=True,
                                 stop=False)
                nc.tensor.matmul(ps[:], biasrow_s[:, 0:H], ones_s[:],
                                 start=False, stop=True)
                sh = cpool.tile([H, W], dt.float32, tag=f"sh{g}", name=f"sh{g}")
                nc.scalar.copy(sh[:], ps[:])
                sh_s.append(sh)
                pd = ppool_big.tile([H, W], dt.float32, tag="pc", name="pc")
                nc.tensor.matmul(pd[:], dwT_s[:], dy_s[:, cs], start=True,
                                 stop=False)
                nc.tensor.matmul(pd[:], biasrow_s[:, H:2 * H], ones_s[:],
                                 start=False, stop=True)
                dh = cpool.tile([H, W], dt.float32, tag=f"dh{g}", name=f"dh{g}")
                nc.vector.tensor_copy(dh[:], pd[:])
                dh_s.append(dh)

            for g in range(NG):
                cs = colrange(g)
                # U = W_as@sh + W_ad@dh
                pu = ppool_big.tile([H, W], dt.float32, tag="pc", name="pc")
                nc.tensor.matmul(pu[:], wasT_s[:], sh_s[g][:], start=True,
                                 stop=False)
                nc.tensor.matmul(pu[:], wadT_s[:], dh_s[g][:], start=False,
                                 stop=True)
                nc.scalar.copy(U_s[g][:], pu[:])
                # V = P_s@sh
                pv = ppool_big.tile([H, W], dt.float32, tag="pc", name="pc")
                nc.tensor.matmul(pv[:], wpsT_s[:], sh_s[g][:], start=True,
                                 stop=True)
                nc.vector.tensor_copy(V_s[g][:], pv[:])
                # PS = P_c@sh -> transpose per item into PST
                pp = ppool_big.tile([H, W], dt.float32, tag="pc", name="pc")
                nc.tensor.matmul(pp[:], wpcT_s[:], sh_s[g][:], start=True,
                                 stop=True)
                ps_sb = spool.tile([H, W], dt.float32, tag="ps_sb",
                                   name="ps_sb")
                nc.scalar.copy(ps_sb[:], pp[:])
                for b in range(GB):
                    pt = ppool_big.tile([S, H], dt.float32, tag="pc",
                                        name="pst_t")
                    nc.tensor.transpose(pt[:], ps_sb[:, b * S:(b + 1) * S],
                                        ident_s[:])
                    dstp = PST_s[g][:, b * H:(b + 1) * H]
                    if b % 2 == 0:
                        nc.scalar.copy(dstp, pt[:])
                    else:
                        nc.vector.tensor_copy(dstp, pt[:])
                # Gtab per gate (with biases), then transpose per (gate,item)
                for k in range(3):
                    pg = ppool_big.tile([H, W], dt.float32, tag="pc", name="pc")
                    nc.tensor.matmul(pg[:], w2T_s[:, k * H:(k + 1) * H],
                                     st_s[:, cs], start=True, stop=False)
                    nc.tensor.matmul(pg[:], biasrow_s[:, (2 + k) * H:(3 + k) * H],
                                     ones_s[:], start=False, stop=True)
                    gt_sb = spool.tile([H, W], dt.float32, tag="gt_sb",
                                       name="gt_sb")
                    nc.scalar.copy(gt_sb[:], pg[:])
                    for b in range(GB):
                        pt = ppool_big.tile([S, H], dt.float32, tag="pc",
                                            name="gt_t")
                        nc.tensor.transpose(pt[:], gt_sb[:, b * S:(b + 1) * S],
                                            ident_s[:])
                        dst = GtT_s[g][:, (k * GB + b) * H:(k * GB + b + 1) * H]
                        if b % 2 == 0:
                            nc.scalar.copy(dst, pt[:])
                        else:
                            nc.vector.tensor_copy(dst, pt[:])

            # ---------------- decode loop ----------------
            gcols = [slice(g * GB, (g + 1) * GB) for g in range(NG)]
            psGHQ = [None, None]   # [H, 4*GB]: rz | NB | Q
            oh_t = [None, None]
            lTs_t = [None, None]
            mxr_t = [None, None]

            def gru_init(g):
                pg = ghq_reg[g][0]
                for k in range(2):
                    nc.tensor.matmul(pg[:, k * GB:(k + 1) * GB],
                                     biasrow_s[:, (5 + k) * H:(6 + k) * H],
                                     ones_s[:, 0:GB], start=True, stop=True,
                                     skip_group_check=True)
                nc.tensor.matmul(pg[:, 2 * GB:3 * GB],
                                 biasrow_s[:, 8 * H:9 * H],
                                 ones_s[:, 0:GB], start=True, stop=True,
                                 skip_group_check=True)
                nc.tensor.matmul(pg[:, 3 * GB:4 * GB],
                                 biasrow_s[:, 7 * H:8 * H],
                                 ones_s[:, 0:GB], start=True, stop=True,
                                 skip_group_check=True)
                psGHQ[g] = pg

            def gru(t, g):
                """psGHQ (whh@h + gi) -> gates -> h update; emits psW for
                this step's attention and whh part of psGHQ(t+1)."""
                cs = gcols[g]
                pg = psGHQ[g]
                th = gpool.tile([H, 2 * GB], dt.float32, tag="th", name="th")
                nc.scalar.activation(th[:], pg[:, 0:2 * GB], AF.Tanh,
                                     scale=0.5)
                t1 = gpool.tile([H, GB], dt.float32, tag="t1", name="t1")
                nc.vector.scalar_tensor_tensor(t1[:], th[:, 0:GB], 1.0,
                                               pg[:, 2 * GB:3 * GB],
                                               op0=ALU.add, op1=ALU.mult)
                na = gpool.tile([H, GB], dt.float32, tag="na", name="na")
                nc.vector.tensor_tensor(na[:], t1[:], pg[:, 3 * GB:4 * GB],
                                        op=ALU.add)
                # zh = z*h = (th_z+1)*(h/2); runs in the na/n shadow
                zh = gpool.tile([H, GB], dt.float32, tag="zh", name="zh")
                nc.vector.scalar_tensor_tensor(zh[:], th[:, GB:2 * GB], 1.0,
                                               h2_s[:, cs], op0=ALU.add,
                                               op1=ALU.mult)
                n_s = gpool.tile([H, GB], dt.float32, tag="n", name="n")
                nc.scalar.activation(n_s[:], na[:], AF.Tanh)
                # h' = (1-z)*n + z*h = -0.5*(th_z-1)*n + zh
                t_ = gpool.tile([H, GB], dt.float32, tag="t_", name="t_")
                nc.vector.scalar_tensor_tensor(t_[:], th[:, GB:2 * GB], 1.0,
                                               n_s[:], op0=ALU.subtract,
                                               op1=ALU.mult)
                nc.vector.scalar_tensor_tensor(h_s[:, cs], t_[:], -0.5,
                                               zh[:], op0=ALU.mult,
                                               op1=ALU.add)
                # psW = wrT@h'
                nc.tensor.matmul(pw_r[g], wrT_s[:], h_s[:, cs], start=True,
                                 stop=True, skip_group_check=True)
                nc.vector.tensor_scalar_mul(h2_s[:, cs], h_s[:, cs], 0.5)
                if t < n_steps - 1:
                    pg2 = ghq_reg[g][(t + 1) & 1]
                    for k in range(2):
                        nc.tensor.matmul(pg2[:, k * GB:(k + 1) * GB],
                                         whhT_s[:, k * H:(k + 1) * H],
                                         h_s[:, cs], start=True, stop=False,
                                         skip_group_check=True)
                    nc.tensor.matmul(pg2[:, 2 * GB:3 * GB], whhn05T_s[:],
                                     h_s[:, cs], start=True, stop=False,
                                     skip_group_check=True)
                    nc.tensor.matmul(pg2[:, 2 * GB:3 * GB],
                                     biasrow_s[:, 8 * H:9 * H],
                                     ones_s[:, 0:GB], start=False, stop=True,
                                     skip_group_check=True)
                    psGHQ[g] = pg2
                else:
                    psGHQ[g] = None

            def front(t, g):
                """Attention front: psA -> ea -> attn logits -> exp ->
                context matmuls."""
                pA = ppool_big.tile([H, W], dt.float32, tag="pc", name="pc")
                nc.vector.tensor_tensor(
                    pA[:].rearrange("p (b s) -> p b s", s=S),
                    U_s[g][:].rearrange("p (b s) -> p b s", s=S),
                    pw_r[g].unsqueeze(2).broadcast_to([H, GB, S]),
                    op=ALU.add)
                ea = spool.tile([H, W], dt.float32, tag="ea", name="ea")
                nc.scalar.activation(ea[:], pA[:], AF.Tanh)
                for b in range(GB):
                    nc.tensor.matmul(qt_r[g][:, b:b + 1],
                                     ea[:, b * S:(b + 1) * S],
                                     vecs_s[:, 4:5], start=True, stop=True,
                                     skip_group_check=True)
                qT = spool.tile([S, GB], dt.float32, tag="qT", name="qT")
                nc.scalar.activation(qT[:], qt_r[g], AF.Exp)
                for b in range(GB):
                    nc.tensor.matmul(w2p_r[g][:, b:b + 1],
                                     PST_s[g][:, b * H:(b + 1) * H],
                                     qT[:, b:b + 1], start=True, stop=True,
                                     skip_group_check=True)
                nc.tensor.matmul(z_r[g], ones64_s[:], qT[:], start=True,
                                 stop=True, skip_group_check=True)

            def back(t, g):
                """Attention back: softmax fold -> pointer tanh -> pointer
                logits -> partition max; qP/Z row for logp."""
                rz_s = gpool.tile([H, GB], dt.float32, tag="rz", name="rz")
                nc.vector.reciprocal(rz_s[:], z_r[g])
                w2 = gpool.tile([H, GB], dt.float32, tag="w2", name="w2")
                nc.vector.tensor_tensor(w2[:], w2p_r[g], rz_s[:], op=ALU.mult)
                pP = ppool_big.tile([H, W], dt.float32, tag="pc", name="pc")
                nc.vector.tensor_tensor(
                    pP[:].rearrange("p (b s) -> p b s", s=S),
                    V_s[g][:].rearrange("p (b s) -> p b s", s=S),
                    w2[:].unsqueeze(2).broadcast_to([H, GB, S]),
                    op=ALU.add)
                ep = spool.tile([H, W], dt.float32, tag="ep", name="ep")
                nc.scalar.activation(ep[:], pP[:], AF.Tanh)
                for b in range(GB):
                    nc.tensor.matmul(lt_r[g][:, b:b + 1],
                                     ep[:, b * S:(b + 1) * S],
                                     vecs_s[:, 5:6], start=True, stop=True,
                                     skip_group_check=True)
                qP = spool.tile([S, GB], dt.float32, tag="qP", name="qP")
                nc.scalar.activation(qP[:], lt_r[g], AF.Exp)
                mxr = spool.tile([S, GB], dt.float32, tag="mxr", name="mxr")
                nc.gpsimd.partition_all_reduce(
                    mxr[:], qP[:], channels=S,
                    reduce_op=bass_isa.ReduceOp.max)
                nc.tensor.matmul(zr_r[g][:, t * GB:(t + 1) * GB],
                                 ones64_s[:, 0:1], qP[:], start=True,
                                 stop=True, skip_group_check=True)
                lTs_t[g] = qP
                mxr_t[g] = mxr

            def tail(t, g):
                """Argmax one-hot; gi matmuls into psGHQ(t+1); oi/logp
                bookkeeping (all deps already satisfied here)."""
                oh = spool.tile([S, GB], dt.float32, tag="oh", name="oh")
                nc.vector.tensor_tensor(oh[:], lTs_t[g][:], mxr_t[g][:],
                                        op=ALU.is_equal)
                if t < n_steps - 1:
                    pg2 = psGHQ[g]
                    for k in range(2):
                        for b in range(GB):
                            nc.tensor.matmul(
                                pg2[:, k * GB + b:k * GB + b + 1],
                                GtT_s[g][:, (k * GB + b) * H:(k * GB + b + 1) * H],
                                oh[:, b:b + 1], start=False,
                                stop=(k == 1 and b == GB - 1),
                                skip_group_check=True)
                    for b in range(GB):
                        nc.tensor.matmul(
                            pg2[:, 3 * GB + b:3 * GB + b + 1],
                            GtT_s[g][:, (2 * GB + b) * H:(2 * GB + b + 1) * H],
                            oh[:, b:b + 1], start=(b == 0), stop=(b == GB - 1),
                            skip_group_check=True)
                nc.tensor.matmul(ic_r[g][:, t:t + 1], oh[:],
                                 vecs_s[0:S, 6:7], start=True, stop=True,
                                 skip_group_check=True)
                nc.gpsimd.tensor_copy(mxbuf_s[g][:, t * GB:(t + 1) * GB],
                                      mxr_t[g][0:1, :])

            # software-pipelined emission:
            #   body(t) = tailA(t-1) gruA(t) backB(t-1) frontA(t)
            #             tailB(t-1) gruB(t) backA(t) frontB(t)
            for g in range(NG):
                gru_init(g)
            for t in range(n_steps):
                if t > 0:
                    tail(t - 1, 0)
                gru(t, 0)
                if t > 0:
                    back(t - 1, 1)
                front(t, 0)
                if t > 0:
                    tail(t - 1, 1)
                gru(t, 1)
                back(t, 0)
                front(t, 1)
            tail(n_steps - 1, 0)
            back(n_steps - 1, 1)
            tail(n_steps - 1, 1)

            # ---------------- epilogue ----------------
            ns = n_steps
            for g in range(NG):
                rz2 = spool.tile([1, S * GB], dt.float32, tag="rz2",
                                 name="rz2")
                nc.vector.reciprocal(rz2[:, 0:ns * GB],
                                     zr_r[g][:, 0:ns * GB])
                nc.vector.tensor_copy(oi_s[g][:, 0:ns], ic_r[g][:, 0:ns])
                rat = spool.tile([1, S * GB], dt.float32, tag="lnq",
                                 name="rat")
                nc.vector.tensor_tensor(rat[:, 0:ns * GB],
                                        mxbuf_s[g][:, 0:ns * GB],
                                        rz2[:, 0:ns * GB], op=ALU.mult)
                olp = spool.tile([1, S * GB], dt.float32, tag="olp",
                                 name="olp")
                nc.scalar.activation(olp[:, 0:ns * GB], rat[:, 0:ns * GB],
                                     AF.Ln)
                # olp free order is (t, b); DRAM wants [b, t]
                olp3 = olp[:, 0:ns * GB].rearrange("p (t b) -> p b t", b=GB)
                for b in range(GB):
                    nc.sync.dma_start(
                        out_logp[g * GB + b:g * GB + b + 1, 0:ns],
                        olp3[:, b, :])
                nc.sync.dma_start(out_idx[g * GB:(g + 1) * GB, 0:ns],
                                  oi_s[g][:, 0:ns])

    nc.compile()
    _legalize_waits(nc)
    return nc


def _legalize_waits(nc):
    """Engine instruction structs carry a limited number of sync waits
    (LDWEIGHTS: 1; ACT/DVE/Pool structs are similarly tight). Move extra
    waits onto injected same-engine nops placed immediately before."""
    import concourse.mybir as mybir

    CAPPED = {mybir.EngineType.PE, mybir.EngineType.Activation,
              mybir.EngineType.DVE, mybir.EngineType.Pool}
    blocks = []
    for f in nc.m.functions:
        for blk in f.blocks:
            blocks.append((blk, list(blk.instructions)))
    final = []
    for blk, insts in blocks:
        out = []
        for i in insts:
            si = i.sync_info
            if (i.engine in CAPPED and si is not None and si.on_wait
                    and len(si.on_wait) > 1
                    and type(i).__name__ != "InstNop"):
                for wt in si.on_wait[:-1]:
                    nop = nc.engines[i.engine].nop().ins
                    nop.sync_info = mybir.SyncInfo(on_wait=[wt], on_update=[])
                    out.append(nop)
                i.sync_info = mybir.SyncInfo(on_wait=[si.on_wait[-1]],
                                             on_update=si.on_update)
            out.append(i)
        final.append((blk, out))
    for blk, out in final:
        blk.instructions = out


def _host_prep(inputs):
    """Build per-core input maps (weight prepack + batch sharding)."""
    f32 = np.float32
    st = np.ascontiguousarray(inputs["static"], dtype=f32)    # [B,2,S]
    dy = np.ascontiguousarray(inputs["dynamic"], dtype=f32)
    x0 = np.asarray(inputs["x0"], dtype=f32)
    sw, sb = np.asarray(inputs["static_w"], f32), np.asarray(inputs["static_b"], f32)
    dw, db = np.asarray(inputs["dynamic_w"], f32), np.asarray(inputs["dynamic_b"], f32)
    decw, decb = np.asarray(inputs["decoder_w"], f32), np.asarray(inputs["decoder_b"], f32)
    wih, whh = np.asarray(inputs["gru_wih"], f32), np.asarray(inputs["gru_whh"], f32)
    bih, bhh = np.asarray(inputs["gru_bih"], f32), np.asarray(inputs["gru_bhh"], f32)
    av, aW = np.asarray(inputs["attn_v"], f32), np.asarray(inputs["attn_W"], f32)
    pv, pW = np.asarray(inputs["ptr_v"], f32), np.asarray(inputs["ptr_W"], f32)

    W2 = (wih @ decw).astype(f32)                  # [3H,2]
    gbias = (wih @ decb + bih).astype(f32)         # [3H]
    bias_r = (gbias[0:H] + bhh[0:H]).astype(f32)
    bias_z = (gbias[H:2 * H] + bhh[H:2 * H]).astype(f32)
    bias_n = gbias[2 * H:3 * H].astype(f32)
    bhh_n = bhh[2 * H:3 * H].astype(f32)
    gi0 = (W2 @ x0 + gbias).astype(f32)
    gi0 = gi0 + np.concatenate([bhh[0:2 * H], np.zeros(H, f32)])

    vecs = np.zeros((H, 8), f32)
    vecs[:, 4] = av
    vecs[:, 5] = pv
    vecs[0:S, 6] = np.arange(S, dtype=f32)

    biasrow = np.concatenate(
        [sb, db, bias_r, bias_z, bias_n, gi0, 0.5 * bhh_n]).reshape(1, 9 * H)

    parts = {
        "swT": sw.T, "dwT": dw.T,
        "w2T": np.concatenate([W2[k * H:(k + 1) * H, :].T for k in range(3)],
                              axis=1),
        "wasT": aW[:, 0:H].T, "wadT": aW[:, H:2 * H].T,
        "wpsT": pW[:, 0:H].T, "wpcT": pW[:, H:2 * H].T,
        "wrT": aW[:, 2 * H:3 * H].T,
        "whhT": np.concatenate([whh[k * H:(k + 1) * H, :].T for k in range(3)],
                               axis=1),
        "whhn05T": 0.5 * whh[2 * H:3 * H, :].T,
        "ones64": np.ones((S, H), f32),
        "vecs": vecs, "biasrow": biasrow,
        "ones_row": np.ones((1, W), f32),
        "ident": np.eye(H, dtype=f32),
    }
    packs = {p: np.zeros((CPACK_ROWS[p], CPACK_COLS[p]), f32)
             for p in CPACK_ROWS}
    for nme, arr in parts.items():
        p, c0, w_ = CPACK_LAYOUT[nme]
        arr = np.asarray(arr, f32)
        packs[p][0:arr.shape[0], c0:c0 + w_] = arr

    in_maps = []
    for c in range(NCORES):
        sl = slice(c * BL, (c + 1) * BL)
        pb = packs["b"].copy()
        _, c0, w_ = CPACK_LAYOUT["st"]
        pb[0:2, c0:c0 + w_] = st[sl].transpose(1, 0, 2).reshape(2, BL * S)
        _, c0, w_ = CPACK_LAYOUT["dy"]
        pb[0:2, c0:c0 + w_] = dy[sl].transpose(1, 0, 2).reshape(2, BL * S)
        in_maps.append({"cpack_a": packs["a"], "cpack_b": pb,
                        "cpack_c": packs["c"]})
    return in_maps


def kernel(**inputs):
    _ensure_path()
    from concourse import bass_utils

    if "nc" not in _CACHE:
        _CACHE["nc"] = _build_program()
    nc = _CACHE["nc"]

    in_maps = _host_prep(inputs)
    res = bass_utils.run_bass_kernel_spmd(nc, in_maps, core_ids=list(range(NCORES)))
    ptrs = np.concatenate([r["out_idx"] for r in res.results], axis=0)
    logps = np.concatenate([r["out_logp"] for r in res.results], axis=0)
    return ptrs.astype(np.int32), logps.astype(np.float32)


# revision 24
# speedup vs baseline: 1.0274x; 1.0274x over previous
"""DRL4TSP pointer-network decode on 8 Trainium2 NeuronCores.

Data-parallel over batch (16 items/core, 2 software-pipelined groups of 8).
All parameters replicated; the 64-step greedy decode runs fully on-device.

Structure (per core, fp32 throughout):
  - Hoisted loop-invariants (computed on device by PE):
      U    = W_as@static_h + W_ad@dynamic_h      [H,(b,s)]
      V    = P_s@static_h                        [H,(b,s)]
      PST  = (P_c@static_h) transposed per item  [S,(b,H)]
      GtT  = ((gru_wih@decoder_w)@static + bias) transposed per
             (gate,item)                         [S,(gate,b,H)]
  - Per decode step, the serial chain is split into 4 phases
    (gru / attn-front / attn-back / argmax-tail) and the two groups are
    emitted software-pipelined so every engine's in-order stream always
    has ready work:
      argmax: pointer logits [S,(b)] psum -> gpsimd partition_all_reduce
      (max) -> DVE is_equal one-hot -> next gi via one-hot matmuls
      against GtT (bit-exact gather); ptr index via one-hot @ iota.
      logp = max - ln(sum exp(l)) banked per step, one Ln at the end.
"""

import numpy as np


def _ensure_path():
    import sys

    try:
        import concourse.bass  # noqa: F401
        return
    except ImportError:
        pass
    for p in ("/opt/trn_rl_repo", "/root/.axon_site/_ro/trn_rl_repo"):
        if p not in sys.path:
            sys.path.insert(0, p)
    import concourse.bass  # noqa: F401


B, S, H = 128, 64, 128
NCORES = 8
BL = B // NCORES          # 16 items per core
NG = 2                    # groups per core
GB = BL // NG             # 8 items per group
W = GB * S                # 512 free width per group
F32 = "float32"

# constant packs, split by row count to minimize DMA bytes:
#   pack "a": 128-row tensors; "b": 2-row; "c": 1-row
_CP_PACKS = {
    "a": [("wasT", H), ("wadT", H), ("wpsT", H), ("wpcT", H), ("wrT", H),
          ("whhT", 3 * H), ("whhn05T", H), ("ones64", H), ("vecs", 8),
          ("wpc", H)],
    "b": [("st", BL * S), ("dy", BL * S), ("swT", H), ("dwT", H)],
    "c": [("biasrow", 9 * H), ("ones_row", W)],
    "d": [("stK", NG * S), ("w2blk", 3 * GB * H)],
}
CPACK_ROWS = {"a": H, "b": 2, "c": 1, "d": 17}
CPACK_LAYOUT = {}
CPACK_COLS = {}
for _p, _lst in _CP_PACKS.items():
    _c = 0
    for _n, _w in _lst:
        CPACK_LAYOUT[_n] = (_p, _c, _w)
        _c += _w
    CPACK_COLS[_p] = _c

_CACHE: dict = {}


def _build_program(n_steps: int = S):
    _ensure_path()
    import concourse.bass as bass
    import concourse.bacc as bacc
    import concourse.mybir as mybir
    import concourse.bass_isa as bass_isa
    from concourse.tile import TileContext

    dt = mybir.dt
    AF = mybir.ActivationFunctionType
    ALU = mybir.AluOpType

    nc = bacc.Bacc("TRN2", target_bir_lowering=False, debug=False,
                   enable_asserts=False, num_devices=NCORES)

    # ---------------- DRAM I/O ----------------
    cpk = {p: nc.dram_tensor(f"cpack_{p}", [CPACK_ROWS[p], CPACK_COLS[p]],
                             dt.float32, kind="ExternalInput").ap()
           for p in CPACK_ROWS}
    out_idx = nc.dram_tensor("out_idx", [BL, S], dt.int32,
                             kind="ExternalOutput").ap()
    out_logp = nc.dram_tensor("out_logp", [BL, S], dt.float32,
                              kind="ExternalOutput").ap()

    with TileContext(nc) as tc:
        import contextlib

        ctx = contextlib.ExitStack()
        with ctx:
            cpool = ctx.enter_context(tc.tile_pool(name="consts", bufs=1))
            spool = ctx.enter_context(tc.tile_pool(name="work", bufs=3))
            gpool = ctx.enter_context(tc.tile_pool(name="gru", bufs=3))
            ppool_big = ctx.enter_context(
                tc.tile_pool(name="psbig", bufs=3, space="PSUM"))
            ppool_fix = ctx.enter_context(
                tc.tile_pool(name="psfix", bufs=1, space="PSUM"))

            # ---- load constants (3 DMAs, one per pack) ----
            cp_t = {}
            for p in CPACK_ROWS:
                cp_t[p] = cpool.tile([CPACK_ROWS[p], CPACK_COLS[p]],
                                     dt.float32, tag=f"cp{p}", name=f"cp{p}")
                nc.sync.dma_start(cp_t[p][:], cpk[p])

            def cslice(name, nrows):
                p, c0, w_ = CPACK_LAYOUT[name]
                return cp_t[p][0:nrows, c0:c0 + w_]

            st_s = cslice("st", 2)
            dy_s = cslice("dy", 2)
            swT_s = cslice("swT", 2)
            dwT_s = cslice("dwT", 2)
            wasT_s = cslice("wasT", H)
            wadT_s = cslice("wadT", H)
            wpsT_s = cslice("wpsT", H)
            wpcT_s = cslice("wpcT", H)
            wrT_s = cslice("wrT", H)
            whhT_s = cslice("whhT", H)
            whhn05T_s = cslice("whhn05T", H)
            ones64_s = cslice("ones64", S)
            vecs_s = cslice("vecs", H)
            biasrow_s = cslice("biasrow", 1)
            ones_s = cslice("ones_row", 1)
            wpc_s = cslice("wpc", H)
            stK_s = cslice("stK", 17)
            w2blk_s = cslice("w2blk", 17)

            # biasrow columns: [0:H]=static_b [H:2H]=dynamic_b
            #   [2H:5H]=Gtab gate biases (r,z incl bhh; n = gbias_n)
            #   [5H:8H]=gi0 rows (r,z incl bhh fold; n plain)
            #   [8H:9H]=0.5*bhh_n
            # vecs columns: 4=attn_v 5=ptr_v 6=iota64(rows 0:64)

            # ---- persistent state ----
            h_s = cpool.tile([H, BL], dt.float32, tag="h", name="h")
            nc.vector.memset(h_s[:], 0.0)
            h2_s = cpool.tile([H, BL], dt.float32, tag="h2", name="h2")
            nc.vector.memset(h2_s[:], 0.0)

            U_s = [cpool.tile([H, W], dt.float32, tag=f"U{g}", name=f"U{g}")
                   for g in range(NG)]
            V_s = [cpool.tile([H, W], dt.float32, tag=f"V{g}", name=f"V{g}")
                   for g in range(NG)]
            PST_s = [cpool.tile([S, GB * H], dt.float32, tag=f"PST{g}",
                                name=f"PST{g}") for g in range(NG)]
            GtT_s = [cpool.tile([S, 3 * GB * H], dt.float32, tag=f"GtT{g}",
                                name=f"GtT{g}") for g in range(NG)]
            Zbuf_s = [cpool.tile([1, S * GB], dt.float32, tag=f"Zb{g}",
                                 name=f"Zb{g}") for g in range(NG)]
            mxbuf_s = [cpool.tile([1, S * GB], dt.float32, tag=f"mxb{g}",
                                  name=f"mxb{g}") for g in range(NG)]
            oi_s = [cpool.tile([GB, S], dt.int32, tag=f"oi{g}", name=f"oi{g}")
                    for g in range(NG)]

            # persistent per-group psum scratch (one full bank each):
            #   pw [H,0:8] | qt [0:64,8:16] | w2p [H,16:24] | z [H,24:32]
            #   lt [0:64,32:40] | zr [0:1,40:48] | ic [0:8,48:49]
            fix = [ppool_fix.tile([H, 128], dt.float32, tag=f"fix{g}",
                                  name=f"fix{g}") for g in range(NG)]
            ghq_t = ppool_fix.tile([H, 128], dt.float32, tag="ghq",
                                   name="ghq")
            ghq_reg = [[ghq_t[:, (2 * g + e) * 32:(2 * g + e + 1) * 32]
                        for e in range(2)] for g in range(NG)]
            zbank = [ppool_fix.tile([H, 512], dt.float32, tag=f"zbk{g}",
                                    name=f"zbk{g}") for g in range(NG)]
            pw_r = [fx[:, 0:GB] for fx in fix]
            qt_r = [fx[0:S, GB:2 * GB] for fx in fix]
            w2p_r = [fx[:, 2 * GB:3 * GB] for fx in fix]
            z_r = [fx[:, 3 * GB:4 * GB] for fx in fix]
            lt_r = [fx[0:S, 4 * GB:5 * GB] for fx in fix]
            zr_r = [zb[0:1, :] for zb in zbank]
            ic_r = [zb[64:64 + GB, 0:S] for zb in zbank]

            # ---------------- precompute ----------------
            # PE p-state warm-up: ~10us of back-to-back dummy matmuls so the
            # ramp hits full clock before the real prologue matmuls.
            wrm = cpool.tile([H, 256], dt.float32, tag="wrm", name="wrm")
            nc.vector.memset(wrm[:], 0.0)
            pwu = ppool_big.tile([H, W], dt.float32, tag="pc", name="pc")
            for _ in range(14):
                nc.tensor.matmul(pwu[:, 0:256], wrm[:, 0:H], wrm[:],
                                 start=True, stop=True,
                                 skip_group_check=True)

            def colrange(g):
                return slice(g * W, (g + 1) * W)

            sh_s, dh_s = [], []
            for g in range(NG):
                cs = colrange(g)
                ps = ppool_big.tile([H, W], dt.float32, tag="pc", name="pc")
                nc.tensor.matmul(ps[:], swT_s[:], st_s[:, cs], start=True,
                                 stop=False)
                nc.tensor.matmul(ps[:], biasrow_s[:, 0:H], ones_s[:],
                                 start=False, stop=True)
                sh = cpool.tile([H, W], dt.float32, tag=f"sh{g}", name=f"sh{g}")
                nc.scalar.copy(sh[:], ps[:])
                sh_s.append(sh)
                pd = ppool_big.tile([H, W], dt.float32, tag="pc", name="pc")
                nc.tensor.matmul(pd[:], dwT_s[:], dy_s[:, cs], start=True,
                                 stop=False)
                nc.tensor.matmul(pd[:], biasrow_s[:, H:2 * H], ones_s[:],
                                 start=False, stop=True)
                dh = cpool.tile([H, W], dt.float32, tag=f"dh{g}", name=f"dh{g}")
                nc.vector.tensor_copy(dh[:], pd[:])
                dh_s.append(dh)

            for g in range(NG):
                cs = colrange(g)
                # U = W_as@sh + W_ad@dh
                pu = ppool_big.tile([H, W], dt.float32, tag="pc", name="pc")
                nc.tensor.matmul(pu[:], wasT_s[:], sh_s[g][:], start=True,
                                 stop=False)
                nc.tensor.matmul(pu[:], wadT_s[:], dh_s[g][:], start=False,
                                 stop=True)
                nc.scalar.copy(U_s[g][:], pu[:])
                # V = P_s@sh
                pv = ppool_big.tile([H, W], dt.float32, tag="pc", name="pc")
                nc.tensor.matmul(pv[:], wpsT_s[:], sh_s[g][:], start=True,
                                 stop=True)
                nc.vector.tensor_copy(V_s[g][:], pv[:])
                # PST_b = sh_b.T @ P_c.T via direct matmuls (no transposes)
                for b in range(GB):
                    pt = ppool_big.tile([S, H], dt.float32, tag="pc",
                                        name="pst_t")
                    nc.tensor.matmul(pt[:], sh_s[g][:, b * S:(b + 1) * S],
                                     wpcT_s[:], start=True, stop=True)
                    dstp = PST_s[g][:, b * H:(b + 1) * H]
                    if b % 2 == 0:
                        nc.scalar.copy(dstp, pt[:])
                    else:
                        nc.vector.tensor_copy(dstp, pt[:])
                # GtT per gate via host-packed block-diagonal weights
                for k in range(3):
                    for hh in range(2):
                        pg = ppool_big.tile([S, W], dt.float32, tag="pc",
                                            name="gtT")
                        nc.tensor.matmul(
                            pg[:], stK_s[:, g * S:(g + 1) * S],
                            w2blk_s[:, k * GB * H + hh * W:
                                    k * GB * H + (hh + 1) * W],
                            start=True, stop=True)
                        dst = GtT_s[g][:, k * GB * H + hh * W:
                                       k * GB * H + (hh + 1) * W]
                        if hh == 0:
                            nc.scalar.copy(dst, pg[:])
                        else:
                            nc.vector.tensor_copy(dst, pg[:])

            # ---------------- decode loop ----------------
            gcols = [slice(g * GB, (g + 1) * GB) for g in range(NG)]
            psGHQ = [None, None]   # [H, 4*GB]: rz | NB | Q
            oh_t = [None, None]
            lTs_t = [None, None]
            mxr_t = [None, None]

            def gru_init(g):
                pg = ghq_reg[g][0]
                for k in range(2):
                    nc.tensor.matmul(pg[:, k * GB:(k + 1) * GB],
                                     biasrow_s[:, (5 + k) * H:(6 + k) * H],
                                     ones_s[:, 0:GB], start=True, stop=True,
                                     skip_group_check=True)
                nc.tensor.matmul(pg[:, 2 * GB:3 * GB],
                                 biasrow_s[:, 8 * H:9 * H],
                                 ones_s[:, 0:GB], start=True, stop=True,
                                 skip_group_check=True)
                nc.tensor.matmul(pg[:, 3 * GB:4 * GB],
                                 biasrow_s[:, 7 * H:8 * H],
                                 ones_s[:, 0:GB], start=True, stop=True,
                                 skip_group_check=True)
                psGHQ[g] = pg

            def gru(t, g):
                """psGHQ (whh@h + gi) -> gates -> h update; emits psW for
                this step's attention and whh part of psGHQ(t+1)."""
                cs = gcols[g]
                pg = psGHQ[g]
                th = gpool.tile([H, 2 * GB], dt.float32, tag="th", name="th")
                nc.scalar.activation(th[:], pg[:, 0:2 * GB], AF.Tanh,
                                     scale=0.5)
                t1 = gpool.tile([H, GB], dt.float32, tag="t1", name="t1")
                nc.vector.scalar_tensor_tensor(t1[:], th[:, 0:GB], 1.0,
                                               pg[:, 2 * GB:3 * GB],
                                               op0=ALU.add, op1=ALU.mult)
                na = gpool.tile([H, GB], dt.float32, tag="na", name="na")
                nc.vector.tensor_tensor(na[:], t1[:], pg[:, 3 * GB:4 * GB],
                                        op=ALU.add)
                # zh = z*h = (th_z+1)*(h/2); runs in the na/n shadow
                zh = gpool.tile([H, GB], dt.float32, tag="zh", name="zh")
                nc.vector.scalar_tensor_tensor(zh[:], th[:, GB:2 * GB], 1.0,
                                               h2_s[:, cs], op0=ALU.add,
                                               op1=ALU.mult)
                n_s = gpool.tile([H, GB], dt.float32, tag="n", name="n")
                nc.scalar.activation(n_s[:], na[:], AF.Tanh)
                # h' = (1-z)*n + z*h = -0.5*(th_z-1)*n + zh
                t_ = gpool.tile([H, GB], dt.float32, tag="t_", name="t_")
                nc.vector.scalar_tensor_tensor(t_[:], th[:, GB:2 * GB], 1.0,
                                               n_s[:], op0=ALU.subtract,
                                               op1=ALU.mult)
                nc.vector.scalar_tensor_tensor(h_s[:, cs], t_[:], -0.5,
                                               zh[:], op0=ALU.mult,
                                               op1=ALU.add)
                # psW = wrT@h'
                nc.tensor.matmul(pw_r[g], wrT_s[:], h_s[:, cs], start=True,
                                 stop=True, skip_group_check=True)
                nc.vector.tensor_scalar_mul(h2_s[:, cs], h_s[:, cs], 0.5)
                if t < n_steps - 1:
                    pg2 = ghq_reg[g][(t + 1) & 1]
                    for k in range(2):
                        nc.tensor.matmul(pg2[:, k * GB:(k + 1) * GB],
                                         whhT_s[:, k * H:(k + 1) * H],
                                         h_s[:, cs], start=True, stop=False,
                                         skip_group_check=True)
                    nc.tensor.matmul(pg2[:, 2 * GB:3 * GB], whhn05T_s[:],
                                     h_s[:, cs], start=True, stop=False,
                                     skip_group_check=True)
                    nc.tensor.matmul(pg2[:, 2 * GB:3 * GB],
                                     biasrow_s[:, 8 * H:9 * H],
                                     ones_s[:, 0:GB], start=False, stop=True,
                                     skip_group_check=True)
                    psGHQ[g] = pg2
                else:
                    psGHQ[g] = None

            def front(t, g):
                """Attention front: psA -> ea -> attn logits -> exp ->
                context matmuls."""
                pA = ppool_big.tile([H, W], dt.float32, tag="pc", name="pc")
                nc.vector.tensor_tensor(
                    pA[:].rearrange("p (b s) -> p b s", s=S),
                    U_s[g][:].rearrange("p (b s) -> p b s", s=S),
                    pw_r[g].unsqueeze(2).broadcast_to([H, GB, S]),
                    op=ALU.add)
                ea = spool.tile([H, W], dt.float32, tag="ea", name="ea")
                nc.scalar.activation(ea[:], pA[:], AF.Tanh)
                for b in range(GB):
                    nc.tensor.matmul(qt_r[g][:, b:b + 1],
                                     ea[:, b * S:(b + 1) * S],
                                     vecs_s[:, 4:5], start=True, stop=True,
                                     skip_group_check=True)
                qT = spool.tile([S, GB], dt.float32, tag="qT", name="qT")
                nc.scalar.activation(qT[:], qt_r[g], AF.Exp)
                for b in range(GB):
                    nc.tensor.matmul(w2p_r[g][:, b:b + 1],
                                     PST_s[g][:, b * H:(b + 1) * H],
                                     qT[:, b:b + 1], start=True, stop=True,
                                     skip_group_check=True)
                nc.tensor.matmul(z_r[g], ones64_s[:], qT[:], start=True,
                                 stop=True, skip_group_check=True)

            def back(t, g):
                """Attention back: softmax fold -> pointer tanh -> pointer
                logits -> partition max; qP/Z row for logp."""
                rz_s = gpool.tile([H, GB], dt.float32, tag="rz", name="rz")
                nc.vector.reciprocal(rz_s[:], z_r[g])
                w2 = gpool.tile([H, GB], dt.float32, tag="w2", name="w2")
                nc.vector.tensor_tensor(w2[:], w2p_r[g], rz_s[:], op=ALU.mult)
                pP = ppool_big.tile([H, W], dt.float32, tag="pc", name="pc")
                nc.vector.tensor_tensor(
                    pP[:].rearrange("p (b s) -> p b s", s=S),
                    V_s[g][:].rearrange("p (b s) -> p b s", s=S),
                    w2[:].unsqueeze(2).broadcast_to([H, GB, S]),
                    op=ALU.add)
                ep = spool.tile([H, W], dt.float32, tag="ep", name="ep")
                nc.scalar.activation(ep[:], pP[:], AF.Tanh)
                for b in range(GB):
                    nc.tensor.matmul(lt_r[g][:, b:b + 1],
                                     ep[:, b * S:(b + 1) * S],
                                     vecs_s[:, 5:6], start=True, stop=True,
                                     skip_group_check=True)
                qP = spool.tile([S, GB], dt.float32, tag="qP", name="qP")
                nc.scalar.activation(qP[:], lt_r[g], AF.Exp)
                mxr = spool.tile([S, GB], dt.float32, tag="mxr", name="mxr")
                nc.gpsimd.partition_all_reduce(
                    mxr[:], qP[:], channels=S,
                    reduce_op=bass_isa.ReduceOp.max)
                nc.tensor.matmul(zr_r[g][:, t * GB:(t + 1) * GB],
                                 ones64_s[:, 0:1], qP[:], start=True,
                                 stop=True, skip_group_check=True)
                lTs_t[g] = qP
                mxr_t[g] = mxr

            def tail(t, g):
                """Argmax one-hot; gi matmuls into psGHQ(t+1); oi/logp
                bookkeeping (all deps already satisfied here)."""
                oh = spool.tile([S, GB], dt.float32, tag="oh", name="oh")
                nc.vector.tensor_tensor(oh[:], lTs_t[g][:], mxr_t[g][:],
                                        op=ALU.is_equal)
                if t < n_steps - 1:
                    pg2 = psGHQ[g]
                    for k in range(2):
                        for b in range(GB):
                            nc.tensor.matmul(
                                pg2[:, k * GB + b:k * GB + b + 1],
                                GtT_s[g][:, (k * GB + b) * H:(k * GB + b + 1) * H],
                                oh[:, b:b + 1], start=False,
                                stop=(k == 1 and b == GB - 1),
                                skip_group_check=True)
                    for b in range(GB):
                        nc.tensor.matmul(
                            pg2[:, 3 * GB + b:3 * GB + b + 1],
                            GtT_s[g][:, (2 * GB + b) * H:(2 * GB + b + 1) * H],
                            oh[:, b:b + 1], start=(b == 0), stop=(b == GB - 1),
                            skip_group_check=True)
                nc.tensor.matmul(ic_r[g][:, t:t + 1], oh[:],
                                 vecs_s[0:S, 6:7], start=True, stop=True,
                                 skip_group_check=True)
                nc.gpsimd.tensor_copy(mxbuf_s[g][:, t * GB:(t + 1) * GB],
                                      mxr_t[g][0:1, :])

            # software-pipelined emission:
            #   body(t) = tailA(t-1) gruA(t) backB(t-1) frontA(t)
            #             tailB(t-1) gruB(t) backA(t) frontB(t)
            for g in range(NG):
                gru_init(g)
            for t in range(n_steps):
                if t > 0:
                    tail(t - 1, 0)
                gru(t, 0)
                if t > 0:
                    back(t - 1, 1)
                front(t, 0)
                if t > 0:
                    tail(t - 1, 1)
                gru(t, 1)
                back(t, 0)
                front(t, 1)
            tail(n_steps - 1, 0)
            back(n_steps - 1, 1)
            tail(n_steps - 1, 1)

            # ---------------- epilogue ----------------
            ns = n_steps
            for g in range(NG):
                rz2 = spool.tile([1, S * GB], dt.float32, tag="rz2",
                                 name="rz2")
                nc.vector.reciprocal(rz2[:, 0:ns * GB],
                                     zr_r[g][:, 0:ns * GB])
                nc.vector.tensor_copy(oi_s[g][:, 0:ns], ic_r[g][:, 0:ns])
                rat = spool.tile([1, S * GB], dt.float32, tag="lnq",
                                 name="rat")
                nc.vector.tensor_tensor(rat[:, 0:ns * GB],
                                        mxbuf_s[g][:, 0:ns * GB],
                                        rz2[:, 0:ns * GB], op=ALU.mult)
                olp = spool.tile([1, S * GB], dt.float32, tag="olp",
                                 name="olp")
                nc.scalar.activation(olp[:, 0:ns * GB], rat[:, 0:ns * GB],
                                     AF.Ln)
                # olp free order is (t, b); DRAM wants [b, t]
                olp3 = olp[:, 0:ns * GB].rearrange("p (t b) -> p b t", b=GB)
                for b in range(GB):
                    nc.sync.dma_start(
                        out_logp[g * GB + b:g * GB + b + 1, 0:ns],
                        olp3[:, b, :])
                nc.sync.dma_start(out_idx[g * GB:(g + 1) * GB, 0:ns],
                                  oi_s[g][:, 0:ns])

    nc.compile()
    _legalize_waits(nc)
    return nc


def _legalize_waits(nc):
    """Engine instruction structs carry a limited number of sync waits
    (LDWEIGHTS: 1; ACT/DVE/Pool structs are similarly tight). Move extra
    waits onto injected same-engine nops placed immediately before."""
    import concourse.mybir as mybir

    CAPPED = {mybir.EngineType.PE, mybir.EngineType.Activation,
              mybir.EngineType.DVE, mybir.EngineType.Pool}
    blocks = []
    for f in nc.m.functions:
        for blk in f.blocks:
            blocks.append((blk, list(blk.instructions)))
    final = []
    for blk, insts in blocks:
        out = []
        for i in insts:
            si = i.sync_info
            if (i.engine in CAPPED and si is not None and si.on_wait
                    and len(si.on_wait) > 1
                    and type(i).__name__ != "InstNop"):
                for wt in si.on_wait[:-1]:
                    nop = nc.engines[i.engine].nop().ins
                    nop.sync_info = mybir.SyncInfo(on_wait=[wt], on_update=[])
                    out.append(nop)
                i.sync_info = mybir.SyncInfo(on_wait=[si.on_wait[-1]],
                                             on_update=si.on_update)
            out.append(i)
        final.append((blk, out))
    for blk, out in final:
        blk.instructions = out


def _host_prep(inputs):
    """Build per-core input maps (weight prepack + batch sharding)."""
    f32 = np.float32
    st = np.ascontiguousarray(inputs["static"], dtype=f32)    # [B,2,S]
    dy = np.ascontiguousarray(inputs["dynamic"], dtype=f32)
    x0 = np.asarray(inputs["x0"], dtype=f32)
    sw, sb = np.asarray(inputs["static_w"], f32), np.asarray(inputs["static_b"], f32)
    dw, db = np.asarray(inputs["dynamic_w"], f32), np.asarray(inputs["dynamic_b"], f32)
    decw, decb = np.asarray(inputs["decoder_w"], f32), np.asarray(inputs["decoder_b"], f32)
    wih, whh = np.asarray(inputs["gru_wih"], f32), np.asarray(inputs["gru_whh"], f32)
    bih, bhh = np.asarray(inputs["gru_bih"], f32), np.asarray(inputs["gru_bhh"], f32)
    av, aW = np.asarray(inputs["attn_v"], f32), np.asarray(inputs["attn_W"], f32)
    pv, pW = np.asarray(inputs["ptr_v"], f32), np.asarray(inputs["ptr_W"], f32)

    W2 = (wih @ decw).astype(f32)                  # [3H,2]
    gbias = (wih @ decb + bih).astype(f32)         # [3H]
    bias_r = (gbias[0:H] + bhh[0:H]).astype(f32)
    bias_z = (gbias[H:2 * H] + bhh[H:2 * H]).astype(f32)
    bias_n = gbias[2 * H:3 * H].astype(f32)
    bhh_n = bhh[2 * H:3 * H].astype(f32)
    gi0 = (W2 @ x0 + gbias).astype(f32)
    gi0 = gi0 + np.concatenate([bhh[0:2 * H], np.zeros(H, f32)])

    vecs = np.zeros((H, 8), f32)
    vecs[:, 4] = av
    vecs[:, 5] = pv
    vecs[0:S, 6] = np.arange(S, dtype=f32)

    biasrow = np.concatenate(
        [sb, db, bias_r, bias_z, bias_n, gi0, 0.5 * bhh_n]).reshape(1, 9 * H)

    # block-diagonal W2 for direct GtT construction: rows (b,c) [+ones],
    # cols (b',h): W2[kH+h, c] iff b == b'
    gate_bias = np.stack([bias_r, bias_z, bias_n], 0)      # [3,H]
    w2blk = np.zeros((17, 3 * GB * H), f32)
    for k in range(3):
        for b in range(GB):
            cols = slice((k * GB + b) * H, (k * GB + b + 1) * H)
            w2blk[2 * b:2 * b + 2, cols] = W2[k * H:(k + 1) * H, :].T
            w2blk[16, cols] = gate_bias[k]
    parts = {
        "swT": sw.T, "dwT": dw.T,
        "wasT": aW[:, 0:H].T, "wadT": aW[:, H:2 * H].T,
        "wpsT": pW[:, 0:H].T, "wpcT": pW[:, H:2 * H].T,
        "wrT": aW[:, 2 * H:3 * H].T,
        "whhT": np.concatenate([whh[k * H:(k + 1) * H, :].T for k in range(3)],
                               axis=1),
        "whhn05T": 0.5 * whh[2 * H:3 * H, :].T,
        "ones64": np.ones((S, H), f32),
        "vecs": vecs, "biasrow": biasrow,
        "ones_row": np.ones((1, W), f32),
        "wpc": pW[:, H:2 * H],
        "w2blk": w2blk,
    }
    packs = {p: np.zeros((CPACK_ROWS[p], CPACK_COLS[p]), f32)
             for p in CPACK_ROWS}
    for nme, arr in parts.items():
        p, c0, w_ = CPACK_LAYOUT[nme]
        arr = np.asarray(arr, f32)
        packs[p][0:arr.shape[0], c0:c0 + w_] = arr

    in_maps = []
    for c in range(NCORES):
        sl = slice(c * BL, (c + 1) * BL)
        pb = packs["b"].copy()
        _, c0, w_ = CPACK_LAYOUT["st"]
        pb[0:2, c0:c0 + w_] = st[sl].transpose(1, 0, 2).reshape(2, BL * S)
        _, c0, w_ = CPACK_LAYOUT["dy"]
        pb[0:2, c0:c0 + w_] = dy[sl].transpose(1, 0, 2).reshape(2, BL * S)
        pd_ = packs["d"].copy()
        _, c0, w_ = CPACK_LAYOUT["stK"]
        stc = st[sl]                                     # [BL,2,S]
        for g in range(NG):
            blk = stc[g * GB:(g + 1) * GB]               # [GB,2,S]
            pd_[0:16, c0 + g * S:c0 + (g + 1) * S] = blk.reshape(16, S)
            pd_[16, c0 + g * S:c0 + (g + 1) * S] = 1.0
        in_maps.append({"cpack_a": packs["a"], "cpack_b": pb,
                        "cpack_c": packs["c"], "cpack_d": pd_})
    return in_maps


def kernel(**inputs):
    _ensure_path()
    from concourse import bass_utils

    if "nc" not in _CACHE:
        _CACHE["nc"] = _build_program()
    nc = _CACHE["nc"]

    in_maps = _host_prep(inputs)
    res = bass_utils.run_bass_kernel_spmd(nc, in_maps, core_ids=list(range(NCORES)))
    ptrs = np.concatenate([r["out_idx"] for r in res.results], axis=0)
    logps = np.concatenate([r["out_logp"] for r in res.results], axis=0)
    return ptrs.astype(np.int32), logps.astype(np.float32)


# revision 25
# speedup vs baseline: 1.0290x; 1.0016x over previous
"""DRL4TSP pointer-network decode on 8 Trainium2 NeuronCores.

Data-parallel over batch (16 items/core, 2 software-pipelined groups of 8).
All parameters replicated; the 64-step greedy decode runs fully on-device.

Structure (per core, fp32 throughout):
  - Hoisted loop-invariants (computed on device by PE):
      U    = W_as@static_h + W_ad@dynamic_h      [H,(b,s)]
      V    = P_s@static_h                        [H,(b,s)]
      PST  = (P_c@static_h) transposed per item  [S,(b,H)]
      GtT  = ((gru_wih@decoder_w)@static + bias) transposed per
             (gate,item)                         [S,(gate,b,H)]
  - Per decode step, the serial chain is split into 4 phases
    (gru / attn-front / attn-back / argmax-tail) and the two groups are
    emitted software-pipelined so every engine's in-order stream always
    has ready work:
      argmax: pointer logits [S,(b)] psum -> gpsimd partition_all_reduce
      (max) -> DVE is_equal one-hot -> next gi via one-hot matmuls
      against GtT (bit-exact gather); ptr index via one-hot @ iota.
      logp = max - ln(sum exp(l)) banked per step, one Ln at the end.
"""

import numpy as np


def _ensure_path():
    import sys

    try:
        import concourse.bass  # noqa: F401
        return
    except ImportError:
        pass
    for p in ("/opt/trn_rl_repo", "/root/.axon_site/_ro/trn_rl_repo"):
        if p not in sys.path:
            sys.path.insert(0, p)
    import concourse.bass  # noqa: F401


B, S, H = 128, 64, 128
NCORES = 8
BL = B // NCORES          # 16 items per core
NG = 2                    # groups per core
GB = BL // NG             # 8 items per group
W = GB * S                # 512 free width per group
F32 = "float32"

# constant packs, split by row count to minimize DMA bytes:
#   pack "a": 128-row tensors; "b": 2-row; "c": 1-row
_CP_PACKS = {
    "a": [("wasT", H), ("wadT", H), ("wpsT", H), ("wpcT", H), ("wrT", H),
          ("whhT", 3 * H), ("whhn05T", H), ("ones64", H), ("vecs", 8),
          ("wpc", H)],
    "b": [("st", BL * S), ("dy", BL * S), ("swT", H), ("dwT", H)],
    "c": [("biasrow", 9 * H), ("ones_row", W)],
    "d": [("stK", NG * S), ("w2blk", 3 * GB * H)],
}
CPACK_ROWS = {"a": H, "b": 2, "c": 1, "d": 17}
CPACK_LAYOUT = {}
CPACK_COLS = {}
for _p, _lst in _CP_PACKS.items():
    _c = 0
    for _n, _w in _lst:
        CPACK_LAYOUT[_n] = (_p, _c, _w)
        _c += _w
    CPACK_COLS[_p] = _c

_CACHE: dict = {}


def _build_program(n_steps: int = S):
    _ensure_path()
    import concourse.bass as bass
    import concourse.bacc as bacc
    import concourse.mybir as mybir
    import concourse.bass_isa as bass_isa
    from concourse.tile import TileContext

    dt = mybir.dt
    AF = mybir.ActivationFunctionType
    ALU = mybir.AluOpType

    nc = bacc.Bacc("TRN2", target_bir_lowering=False, debug=False,
                   enable_asserts=False, num_devices=NCORES)

    # ---------------- DRAM I/O ----------------
    cpk = {p: nc.dram_tensor(f"cpack_{p}", [CPACK_ROWS[p], CPACK_COLS[p]],
                             dt.float32, kind="ExternalInput").ap()
           for p in CPACK_ROWS}
    out_idx = nc.dram_tensor("out_idx", [BL, S], dt.int32,
                             kind="ExternalOutput").ap()
    out_logp = nc.dram_tensor("out_logp", [BL, S], dt.float32,
                              kind="ExternalOutput").ap()

    with TileContext(nc) as tc:
        import contextlib

        ctx = contextlib.ExitStack()
        with ctx:
            cpool = ctx.enter_context(tc.tile_pool(name="consts", bufs=1))
            spool = ctx.enter_context(tc.tile_pool(name="work", bufs=3))
            gpool = ctx.enter_context(tc.tile_pool(name="gru", bufs=3))
            ppool_big = ctx.enter_context(
                tc.tile_pool(name="psbig", bufs=3, space="PSUM"))
            ppool_fix = ctx.enter_context(
                tc.tile_pool(name="psfix", bufs=1, space="PSUM"))

            # ---- load constants (3 DMAs, one per pack) ----
            cp_t = {}
            for p in CPACK_ROWS:
                cp_t[p] = cpool.tile([CPACK_ROWS[p], CPACK_COLS[p]],
                                     dt.float32, tag=f"cp{p}", name=f"cp{p}")
                nc.sync.dma_start(cp_t[p][:], cpk[p])

            def cslice(name, nrows):
                p, c0, w_ = CPACK_LAYOUT[name]
                return cp_t[p][0:nrows, c0:c0 + w_]

            st_s = cslice("st", 2)
            dy_s = cslice("dy", 2)
            swT_s = cslice("swT", 2)
            dwT_s = cslice("dwT", 2)
            wasT_s = cslice("wasT", H)
            wadT_s = cslice("wadT", H)
            wpsT_s = cslice("wpsT", H)
            wpcT_s = cslice("wpcT", H)
            wrT_s = cslice("wrT", H)
            whhT_s = cslice("whhT", H)
            whhn05T_s = cslice("whhn05T", H)
            ones64_s = cslice("ones64", S)
            vecs_s = cslice("vecs", H)
            biasrow_s = cslice("biasrow", 1)
            ones_s = cslice("ones_row", 1)
            wpc_s = cslice("wpc", H)
            stK_s = cslice("stK", 17)
            w2blk_s = cslice("w2blk", 17)

            # biasrow columns: [0:H]=static_b [H:2H]=dynamic_b
            #   [2H:5H]=Gtab gate biases (r,z incl bhh; n = gbias_n)
            #   [5H:8H]=gi0 rows (r,z incl bhh fold; n plain)
            #   [8H:9H]=0.5*bhh_n
            # vecs columns: 4=attn_v 5=ptr_v 6=iota64(rows 0:64)

            # ---- persistent state ----
            h_s = cpool.tile([H, BL], dt.float32, tag="h", name="h")
            nc.vector.memset(h_s[:], 0.0)
            h2_s = cpool.tile([H, BL], dt.float32, tag="h2", name="h2")
            nc.vector.memset(h2_s[:], 0.0)

            U_s = [cpool.tile([H, W], dt.float32, tag=f"U{g}", name=f"U{g}")
                   for g in range(NG)]
            V_s = [cpool.tile([H, W], dt.float32, tag=f"V{g}", name=f"V{g}")
                   for g in range(NG)]
            PST_s = [cpool.tile([S, GB * H], dt.float32, tag=f"PST{g}",
                                name=f"PST{g}") for g in range(NG)]
            GtT_s = [cpool.tile([S, 3 * GB * H], dt.float32, tag=f"GtT{g}",
                                name=f"GtT{g}") for g in range(NG)]
            Zbuf_s = [cpool.tile([1, S * GB], dt.float32, tag=f"Zb{g}",
                                 name=f"Zb{g}") for g in range(NG)]
            mxbuf_s = [cpool.tile([1, S * GB], dt.float32, tag=f"mxb{g}",
                                  name=f"mxb{g}") for g in range(NG)]
            oi_s = [cpool.tile([GB, S], dt.int32, tag=f"oi{g}", name=f"oi{g}")
                    for g in range(NG)]

            # persistent per-group psum scratch (one full bank each):
            #   pw [H,0:8] | qt [0:64,8:16] | w2p [H,16:24] | z [H,24:32]
            #   lt [0:64,32:40] | zr [0:1,40:48] | ic [0:8,48:49]
            fix = [ppool_fix.tile([H, 128], dt.float32, tag=f"fix{g}",
                                  name=f"fix{g}") for g in range(NG)]
            ghq_t = ppool_fix.tile([H, 128], dt.float32, tag="ghq",
                                   name="ghq")
            ghq_reg = [[ghq_t[:, (2 * g + e) * 32:(2 * g + e + 1) * 32]
                        for e in range(2)] for g in range(NG)]
            zbank = [ppool_fix.tile([H, 512], dt.float32, tag=f"zbk{g}",
                                    name=f"zbk{g}") for g in range(NG)]
            pw_r = [fx[:, 0:GB] for fx in fix]
            qt_r = [fx[0:S, GB:2 * GB] for fx in fix]
            w2p_r = [fx[:, 2 * GB:3 * GB] for fx in fix]
            z_r = [fx[:, 3 * GB:4 * GB] for fx in fix]
            lt_r = [fx[0:S, 4 * GB:5 * GB] for fx in fix]
            zr_r = [zb[0:1, :] for zb in zbank]
            ic_r = [zb[64:64 + GB, 0:S] for zb in zbank]

            # ---------------- precompute ----------------
            def colrange(g):
                return slice(g * W, (g + 1) * W)

            sh_s, dh_s = [], []
            for g in range(NG):
                cs = colrange(g)
                ps = ppool_big.tile([H, W], dt.float32, tag="pc", name="pc")
                nc.tensor.matmul(ps[:], swT_s[:], st_s[:, cs], start=True,
                                 stop=False)
                nc.tensor.matmul(ps[:], biasrow_s[:, 0:H], ones_s[:],
                                 start=False, stop=True)
                sh = cpool.tile([H, W], dt.float32, tag=f"sh{g}", name=f"sh{g}")
                nc.scalar.copy(sh[:], ps[:])
                sh_s.append(sh)
                pd = ppool_big.tile([H, W], dt.float32, tag="pc", name="pc")
                nc.tensor.matmul(pd[:], dwT_s[:], dy_s[:, cs], start=True,
                                 stop=False)
                nc.tensor.matmul(pd[:], biasrow_s[:, H:2 * H], ones_s[:],
                                 start=False, stop=True)
                dh = cpool.tile([H, W], dt.float32, tag=f"dh{g}", name=f"dh{g}")
                nc.vector.tensor_copy(dh[:], pd[:])
                dh_s.append(dh)

            for g in range(NG):
                cs = colrange(g)
                # U = W_as@sh + W_ad@dh
                pu = ppool_big.tile([H, W], dt.float32, tag="pc", name="pc")
                nc.tensor.matmul(pu[:], wasT_s[:], sh_s[g][:], start=True,
                                 stop=False)
                nc.tensor.matmul(pu[:], wadT_s[:], dh_s[g][:], start=False,
                                 stop=True)
                nc.scalar.copy(U_s[g][:], pu[:])
                # V = P_s@sh
                pv = ppool_big.tile([H, W], dt.float32, tag="pc", name="pc")
                nc.tensor.matmul(pv[:], wpsT_s[:], sh_s[g][:], start=True,
                                 stop=True)
                nc.vector.tensor_copy(V_s[g][:], pv[:])
                # PST_b = sh_b.T @ P_c.T via direct matmuls (no transposes)
                for b in range(GB):
                    pt = ppool_big.tile([S, H], dt.float32, tag="pc",
                                        name="pst_t")
                    nc.tensor.matmul(pt[:], sh_s[g][:, b * S:(b + 1) * S],
                                     wpcT_s[:], start=True, stop=True)
                    dstp = PST_s[g][:, b * H:(b + 1) * H]
                    if b % 2 == 0:
                        nc.scalar.copy(dstp, pt[:])
                    else:
                        nc.vector.tensor_copy(dstp, pt[:])
                # GtT per gate via host-packed block-diagonal weights
                for k in range(3):
                    for hh in range(2):
                        pg = ppool_big.tile([S, W], dt.float32, tag="pc",
                                            name="gtT")
                        nc.tensor.matmul(
                            pg[:], stK_s[:, g * S:(g + 1) * S],
                            w2blk_s[:, k * GB * H + hh * W:
                                    k * GB * H + (hh + 1) * W],
                            start=True, stop=True)
                        dst = GtT_s[g][:, k * GB * H + hh * W:
                                       k * GB * H + (hh + 1) * W]
                        if hh == 0:
                            nc.scalar.copy(dst, pg[:])
                        else:
                            nc.vector.tensor_copy(dst, pg[:])

            # ---------------- decode loop ----------------
            gcols = [slice(g * GB, (g + 1) * GB) for g in range(NG)]
            psGHQ = [None, None]   # [H, 4*GB]: rz | NB | Q
            oh_t = [None, None]
            lTs_t = [None, None]
            mxr_t = [None, None]

            def gru_init(g):
                pg = ghq_reg[g][0]
                for k in range(2):
                    nc.tensor.matmul(pg[:, k * GB:(k + 1) * GB],
                                     biasrow_s[:, (5 + k) * H:(6 + k) * H],
                                     ones_s[:, 0:GB], start=True, stop=True,
                                     skip_group_check=True)
                nc.tensor.matmul(pg[:, 2 * GB:3 * GB],
                                 biasrow_s[:, 8 * H:9 * H],
                                 ones_s[:, 0:GB], start=True, stop=True,
                                 skip_group_check=True)
                nc.tensor.matmul(pg[:, 3 * GB:4 * GB],
                                 biasrow_s[:, 7 * H:8 * H],
                                 ones_s[:, 0:GB], start=True, stop=True,
                                 skip_group_check=True)
                psGHQ[g] = pg

            def gru(t, g):
                """psGHQ (whh@h + gi) -> gates -> h update; emits psW for
                this step's attention and whh part of psGHQ(t+1)."""
                cs = gcols[g]
                pg = psGHQ[g]
                th = gpool.tile([H, 2 * GB], dt.float32, tag="th", name="th")
                nc.scalar.activation(th[:], pg[:, 0:2 * GB], AF.Tanh,
                                     scale=0.5)
                t1 = gpool.tile([H, GB], dt.float32, tag="t1", name="t1")
                nc.vector.scalar_tensor_tensor(t1[:], th[:, 0:GB], 1.0,
                                               pg[:, 2 * GB:3 * GB],
                                               op0=ALU.add, op1=ALU.mult)
                na = gpool.tile([H, GB], dt.float32, tag="na", name="na")
                nc.vector.tensor_tensor(na[:], t1[:], pg[:, 3 * GB:4 * GB],
                                        op=ALU.add)
                # zh = z*h = (th_z+1)*(h/2); runs in the na/n shadow
                zh = gpool.tile([H, GB], dt.float32, tag="zh", name="zh")
                nc.vector.scalar_tensor_tensor(zh[:], th[:, GB:2 * GB], 1.0,
                                               h2_s[:, cs], op0=ALU.add,
                                               op1=ALU.mult)
                n_s = gpool.tile([H, GB], dt.float32, tag="n", name="n")
                nc.scalar.activation(n_s[:], na[:], AF.Tanh)
                # h' = (1-z)*n + z*h = -0.5*(th_z-1)*n + zh
                t_ = gpool.tile([H, GB], dt.float32, tag="t_", name="t_")
                nc.vector.scalar_tensor_tensor(t_[:], th[:, GB:2 * GB], 1.0,
                                               n_s[:], op0=ALU.subtract,
                                               op1=ALU.mult)
                nc.vector.scalar_tensor_tensor(h_s[:, cs], t_[:], -0.5,
                                               zh[:], op0=ALU.mult,
                                               op1=ALU.add)
                # psW = wrT@h'
                nc.tensor.matmul(pw_r[g], wrT_s[:], h_s[:, cs], start=True,
                                 stop=True, skip_group_check=True)
                nc.vector.tensor_scalar_mul(h2_s[:, cs], h_s[:, cs], 0.5)
                if t < n_steps - 1:
                    pg2 = ghq_reg[g][(t + 1) & 1]
                    for k in range(2):
                        nc.tensor.matmul(pg2[:, k * GB:(k + 1) * GB],
                                         whhT_s[:, k * H:(k + 1) * H],
                                         h_s[:, cs], start=True, stop=False,
                                         skip_group_check=True)
                    nc.tensor.matmul(pg2[:, 2 * GB:3 * GB], whhn05T_s[:],
                                     h_s[:, cs], start=True, stop=False,
                                     skip_group_check=True)
                    nc.tensor.matmul(pg2[:, 2 * GB:3 * GB],
                                     biasrow_s[:, 8 * H:9 * H],
                                     ones_s[:, 0:GB], start=False, stop=True,
                                     skip_group_check=True)
                    psGHQ[g] = pg2
                else:
                    psGHQ[g] = None

            def front(t, g):
                """Attention front: psA -> ea -> attn logits -> exp ->
                context matmuls."""
                pA = ppool_big.tile([H, W], dt.float32, tag="pc", name="pc")
                nc.vector.tensor_tensor(
                    pA[:].rearrange("p (b s) -> p b s", s=S),
                    U_s[g][:].rearrange("p (b s) -> p b s", s=S),
                    pw_r[g].unsqueeze(2).broadcast_to([H, GB, S]),
                    op=ALU.add)
                ea = spool.tile([H, W], dt.float32, tag="ea", name="ea")
                nc.scalar.activation(ea[:], pA[:], AF.Tanh)
                for b in range(GB):
                    nc.tensor.matmul(qt_r[g][:, b:b + 1],
                                     ea[:, b * S:(b + 1) * S],
                                     vecs_s[:, 4:5], start=True, stop=True,
                                     skip_group_check=True)
                qT = spool.tile([S, GB], dt.float32, tag="qT", name="qT")
                nc.scalar.activation(qT[:], qt_r[g], AF.Exp)
                for b in range(GB):
                    nc.tensor.matmul(w2p_r[g][:, b:b + 1],
                                     PST_s[g][:, b * H:(b + 1) * H],
                                     qT[:, b:b + 1], start=True, stop=True,
                                     skip_group_check=True)
                nc.tensor.matmul(z_r[g], ones64_s[:], qT[:], start=True,
                                 stop=True, skip_group_check=True)

            def back(t, g):
                """Attention back: softmax fold -> pointer tanh -> pointer
                logits -> partition max; qP/Z row for logp."""
                rz_s = gpool.tile([H, GB], dt.float32, tag="rz", name="rz")
                nc.vector.reciprocal(rz_s[:], z_r[g])
                w2 = gpool.tile([H, GB], dt.float32, tag="w2", name="w2")
                nc.vector.tensor_tensor(w2[:], w2p_r[g], rz_s[:], op=ALU.mult)
                pP = ppool_big.tile([H, W], dt.float32, tag="pc", name="pc")
                nc.vector.tensor_tensor(
                    pP[:].rearrange("p (b s) -> p b s", s=S),
                    V_s[g][:].rearrange("p (b s) -> p b s", s=S),
                    w2[:].unsqueeze(2).broadcast_to([H, GB, S]),
                    op=ALU.add)
                ep = spool.tile([H, W], dt.float32, tag="ep", name="ep")
                nc.scalar.activation(ep[:], pP[:], AF.Tanh)
                for b in range(GB):
                    nc.tensor.matmul(lt_r[g][:, b:b + 1],
                                     ep[:, b * S:(b + 1) * S],
                                     vecs_s[:, 5:6], start=True, stop=True,
                                     skip_group_check=True)
                qP = spool.tile([S, GB], dt.float32, tag="qP", name="qP")
                nc.scalar.activation(qP[:], lt_r[g], AF.Exp)
                mxr = spool.tile([S, GB], dt.float32, tag="mxr", name="mxr")
                nc.gpsimd.partition_all_reduce(
                    mxr[:], qP[:], channels=S,
                    reduce_op=bass_isa.ReduceOp.max)
                nc.tensor.matmul(zr_r[g][:, t * GB:(t + 1) * GB],
                                 ones64_s[:, 0:1], qP[:], start=True,
                                 stop=True, skip_group_check=True)
                lTs_t[g] = qP
                mxr_t[g] = mxr

            def tail(t, g):
                """Argmax one-hot; gi matmuls into psGHQ(t+1); oi/logp
                bookkeeping (all deps already satisfied here)."""
                oh = spool.tile([S, GB], dt.float32, tag="oh", name="oh")
                nc.vector.tensor_tensor(oh[:], lTs_t[g][:], mxr_t[g][:],
                                        op=ALU.is_equal)
                if t < n_steps - 1:
                    pg2 = psGHQ[g]
                    for k in range(2):
                        for b in range(GB):
                            nc.tensor.matmul(
                                pg2[:, k * GB + b:k * GB + b + 1],
                                GtT_s[g][:, (k * GB + b) * H:(k * GB + b + 1) * H],
                                oh[:, b:b + 1], start=False,
                                stop=(k == 1 and b == GB - 1),
                                skip_group_check=True)
                    for b in range(GB):
                        nc.tensor.matmul(
                            pg2[:, 3 * GB + b:3 * GB + b + 1],
                            GtT_s[g][:, (2 * GB + b) * H:(2 * GB + b + 1) * H],
                            oh[:, b:b + 1], start=(b == 0), stop=(b == GB - 1),
                            skip_group_check=True)
                nc.tensor.matmul(ic_r[g][:, t:t + 1], oh[:],
                                 vecs_s[0:S, 6:7], start=True, stop=True,
                                 skip_group_check=True)
                nc.gpsimd.tensor_copy(mxbuf_s[g][:, t * GB:(t + 1) * GB],
                                      mxr_t[g][0:1, :])

            # software-pipelined emission:
            #   body(t) = tailA(t-1) gruA(t) backB(t-1) frontA(t)
            #             tailB(t-1) gruB(t) backA(t) frontB(t)
            for g in range(NG):
                gru_init(g)
            for t in range(n_steps):
                if t > 0:
                    tail(t - 1, 0)
                gru(t, 0)
                if t > 0:
                    back(t - 1, 1)
                front(t, 0)
                if t > 0:
                    tail(t - 1, 1)
                gru(t, 1)
                back(t, 0)
                front(t, 1)
            tail(n_steps - 1, 0)
            back(n_steps - 1, 1)
            tail(n_steps - 1, 1)

            # ---------------- epilogue ----------------
            ns = n_steps
            for g in range(NG):
                rz2 = spool.tile([1, S * GB], dt.float32, tag="rz2",
                                 name="rz2")
                nc.vector.reciprocal(rz2[:, 0:ns * GB],
                                     zr_r[g][:, 0:ns * GB])
                nc.vector.tensor_copy(oi_s[g][:, 0:ns], ic_r[g][:, 0:ns])
                rat = spool.tile([1, S * GB], dt.float32, tag="lnq",
                                 name="rat")
                nc.vector.tensor_tensor(rat[:, 0:ns * GB],
                                        mxbuf_s[g][:, 0:ns * GB],
                                        rz2[:, 0:ns * GB], op=ALU.mult)
                olp = spool.tile([1, S * GB], dt.float32, tag="olp",
                                 name="olp")
                nc.scalar.activation(olp[:, 0:ns * GB], rat[:, 0:ns * GB],
                                     AF.Ln)
                # olp free order is (t, b); DRAM wants [b, t]
                olp3 = olp[:, 0:ns * GB].rearrange("p (t b) -> p b t", b=GB)
                for b in range(GB):
                    nc.sync.dma_start(
                        out_logp[g * GB + b:g * GB + b + 1, 0:ns],
                        olp3[:, b, :])
                nc.sync.dma_start(out_idx[g * GB:(g + 1) * GB, 0:ns],
                                  oi_s[g][:, 0:ns])

    nc.compile()
    _legalize_waits(nc)
    return nc


def _legalize_waits(nc):
    """Engine instruction structs carry a limited number of sync waits
    (LDWEIGHTS: 1; ACT/DVE/Pool structs are similarly tight). Move extra
    waits onto injected same-engine nops placed immediately before."""
    import concourse.mybir as mybir

    CAPPED = {mybir.EngineType.PE, mybir.EngineType.Activation,
              mybir.EngineType.DVE, mybir.EngineType.Pool}
    blocks = []
    for f in nc.m.functions:
        for blk in f.blocks:
            blocks.append((blk, list(blk.instructions)))
    final = []
    for blk, insts in blocks:
        out = []
        for i in insts:
            si = i.sync_info
            if (i.engine in CAPPED and si is not None and si.on_wait
                    and len(si.on_wait) > 1
                    and type(i).__name__ != "InstNop"):
                for wt in si.on_wait[:-1]:
                    nop = nc.engines[i.engine].nop().ins
                    nop.sync_info = mybir.SyncInfo(on_wait=[wt], on_update=[])
                    out.append(nop)
                i.sync_info = mybir.SyncInfo(on_wait=[si.on_wait[-1]],
                                             on_update=si.on_update)
            out.append(i)
        final.append((blk, out))
    for blk, out in final:
        blk.instructions = out


def _host_prep(inputs):
    """Build per-core input maps (weight prepack + batch sharding)."""
    f32 = np.float32
    st = np.ascontiguousarray(inputs["static"], dtype=f32)    # [B,2,S]
    dy = np.ascontiguousarray(inputs["dynamic"], dtype=f32)
    x0 = np.asarray(inputs["x0"], dtype=f32)
    sw, sb = np.asarray(inputs["static_w"], f32), np.asarray(inputs["static_b"], f32)
    dw, db = np.asarray(inputs["dynamic_w"], f32), np.asarray(inputs["dynamic_b"], f32)
    decw, decb = np.asarray(inputs["decoder_w"], f32), np.asarray(inputs["decoder_b"], f32)
    wih, whh = np.asarray(inputs["gru_wih"], f32), np.asarray(inputs["gru_whh"], f32)
    bih, bhh = np.asarray(inputs["gru_bih"], f32), np.asarray(inputs["gru_bhh"], f32)
    av, aW = np.asarray(inputs["attn_v"], f32), np.asarray(inputs["attn_W"], f32)
    pv, pW = np.asarray(inputs["ptr_v"], f32), np.asarray(inputs["ptr_W"], f32)

    W2 = (wih @ decw).astype(f32)                  # [3H,2]
    gbias = (wih @ decb + bih).astype(f32)         # [3H]
    bias_r = (gbias[0:H] + bhh[0:H]).astype(f32)
    bias_z = (gbias[H:2 * H] + bhh[H:2 * H]).astype(f32)
    bias_n = gbias[2 * H:3 * H].astype(f32)
    bhh_n = bhh[2 * H:3 * H].astype(f32)
    gi0 = (W2 @ x0 + gbias).astype(f32)
    gi0 = gi0 + np.concatenate([bhh[0:2 * H], np.zeros(H, f32)])

    vecs = np.zeros((H, 8), f32)
    vecs[:, 4] = av
    vecs[:, 5] = pv
    vecs[0:S, 6] = np.arange(S, dtype=f32)

    biasrow = np.concatenate(
        [sb, db, bias_r, bias_z, bias_n, gi0, 0.5 * bhh_n]).reshape(1, 9 * H)

    # block-diagonal W2 for direct GtT construction: rows (b,c) [+ones],
    # cols (b',h): W2[kH+h, c] iff b == b'
    gate_bias = np.stack([bias_r, bias_z, bias_n], 0)      # [3,H]
    w2blk = np.zeros((17, 3 * GB * H), f32)
    for k in range(3):
        for b in range(GB):
            cols = slice((k * GB + b) * H, (k * GB + b + 1) * H)
            w2blk[2 * b:2 * b + 2, cols] = W2[k * H:(k + 1) * H, :].T
            w2blk[16, cols] = gate_bias[k]
    parts = {
        "swT": sw.T, "dwT": dw.T,
        "wasT": aW[:, 0:H].T, "wadT": aW[:, H:2 * H].T,
        "wpsT": pW[:, 0:H].T, "wpcT": pW[:, H:2 * H].T,
        "wrT": aW[:, 2 * H:3 * H].T,
        "whhT": np.concatenate([whh[k * H:(k + 1) * H, :].T for k in range(3)],
                               axis=1),
        "whhn05T": 0.5 * whh[2 * H:3 * H, :].T,
        "ones64": np.ones((S, H), f32),
        "vecs": vecs, "biasrow": biasrow,
        "ones_row": np.ones((1, W), f32),
        "wpc": pW[:, H:2 * H],
        "w2blk": w2blk,
    }
    packs = {p: np.zeros((CPACK_ROWS[p], CPACK_COLS[p]), f32)
             for p in CPACK_ROWS}
    for nme, arr in parts.items():
        p, c0, w_ = CPACK_LAYOUT[nme]
        arr = np.asarray(arr, f32)
        packs[p][0:arr.shape[0], c0:c0 + w_] = arr

    in_maps = []
    for c in range(NCORES):
        sl = slice(c * BL, (c + 1) * BL)
        pb = packs["b"].copy()
        _, c0, w_ = CPACK_LAYOUT["st"]
        pb[0:2, c0:c0 + w_] = st[sl].transpose(1, 0, 2).reshape(2, BL * S)
        _, c0, w_ = CPACK_LAYOUT["dy"]
        pb[0:2, c0:c0 + w_] = dy[sl].transpose(1, 0, 2).reshape(2, BL * S)
        pd_ = packs["d"].copy()
        _, c0, w_ = CPACK_LAYOUT["stK"]
        stc = st[sl]                                     # [BL,2,S]
        for g in range(NG):
            blk = stc[g * GB:(g + 1) * GB]               # [GB,2,S]
            pd_[0:16, c0 + g * S:c0 + (g + 1) * S] = blk.reshape(16, S)
            pd_[16, c0 + g * S:c0 + (g + 1) * S] = 1.0
        in_maps.append({"cpack_a": packs["a"], "cpack_b": pb,
                        "cpack_c": packs["c"], "cpack_d": pd_})
    return in_maps


def kernel(**inputs):
    _ensure_path()
    from concourse import bass_utils

    if "nc" not in _CACHE:
        _CACHE["nc"] = _build_program()
    nc = _CACHE["nc"]

    in_maps = _host_prep(inputs)
    res = bass_utils.run_bass_kernel_spmd(nc, in_maps, core_ids=list(range(NCORES)))
    ptrs = np.concatenate([r["out_idx"] for r in res.results], axis=0)
    logps = np.concatenate([r["out_logp"] for r in res.results], axis=0)
    return ptrs.astype(np.int32), logps.astype(np.float32)


# revision 26
# speedup vs baseline: 1.0336x; 1.0044x over previous
"""DRL4TSP pointer-network decode on 8 Trainium2 NeuronCores.

Data-parallel over batch (16 items/core, 2 software-pipelined groups of 8).
All parameters replicated; the 64-step greedy decode runs fully on-device.

Structure (per core, fp32 throughout):
  - Hoisted loop-invariants (computed on device by PE):
      U    = W_as@static_h + W_ad@dynamic_h      [H,(b,s)]
      V    = P_s@static_h                        [H,(b,s)]
      PST  = (P_c@static_h) transposed per item  [S,(b,H)]
      GtT  = ((gru_wih@decoder_w)@static + bias) transposed per
             (gate,item)                         [S,(gate,b,H)]
  - Per decode step, the serial chain is split into 4 phases
    (gru / attn-front / attn-back / argmax-tail) and the two groups are
    emitted software-pipelined so every engine's in-order stream always
    has ready work:
      argmax: pointer logits [S,(b)] psum -> gpsimd partition_all_reduce
      (max) -> DVE is_equal one-hot -> next gi via one-hot matmuls
      against GtT (bit-exact gather); ptr index via one-hot @ iota.
      logp = max - ln(sum exp(l)) banked per step, one Ln at the end.
"""

import numpy as np


def _ensure_path():
    import sys

    try:
        import concourse.bass  # noqa: F401
        return
    except ImportError:
        pass
    for p in ("/opt/trn_rl_repo", "/root/.axon_site/_ro/trn_rl_repo"):
        if p not in sys.path:
            sys.path.insert(0, p)
    import concourse.bass  # noqa: F401


B, S, H = 128, 64, 128
NCORES = 8
BL = B // NCORES          # 16 items per core
NG = 2                    # groups per core
GB = BL // NG             # 8 items per group
W = GB * S                # 512 free width per group
F32 = "float32"

# constant packs, split by row count to minimize DMA bytes:
#   pack "a": 128-row tensors; "b": 2-row; "c": 1-row
_CP_PACKS = {
    "a": [("wasT", H), ("wadT", H), ("wpsT", H), ("wpcT", H), ("wrT", H),
          ("whhT", 3 * H), ("whhn05T", H), ("ones64", H), ("vecs", 8),
          ("wpc", H)],
    "b": [("st", BL * S), ("dy", BL * S), ("swT", H), ("dwT", H)],
    "c": [("biasrow", 9 * H), ("ones_row", W)],
    "d": [("stK", NG * S), ("w2blk", 3 * GB * H)],
}
CPACK_ROWS = {"a": H, "b": 2, "c": 1, "d": 17}
CPACK_LAYOUT = {}
CPACK_COLS = {}
for _p, _lst in _CP_PACKS.items():
    _c = 0
    for _n, _w in _lst:
        CPACK_LAYOUT[_n] = (_p, _c, _w)
        _c += _w
    CPACK_COLS[_p] = _c

_CACHE: dict = {}


def _build_program(n_steps: int = S):
    _ensure_path()
    import concourse.bass as bass
    import concourse.bacc as bacc
    import concourse.mybir as mybir
    import concourse.bass_isa as bass_isa
    from concourse.tile import TileContext

    dt = mybir.dt
    AF = mybir.ActivationFunctionType
    ALU = mybir.AluOpType

    nc = bacc.Bacc("TRN2", target_bir_lowering=False, debug=False,
                   enable_asserts=False, num_devices=NCORES)

    # ---------------- DRAM I/O ----------------
    cpk = {p: nc.dram_tensor(f"cpack_{p}", [CPACK_ROWS[p], CPACK_COLS[p]],
                             dt.float32, kind="ExternalInput").ap()
           for p in CPACK_ROWS}
    out_idx = nc.dram_tensor("out_idx", [BL, S], dt.int32,
                             kind="ExternalOutput").ap()
    out_logp = nc.dram_tensor("out_logp", [BL, S], dt.float32,
                              kind="ExternalOutput").ap()

    with TileContext(nc) as tc:
        import contextlib

        ctx = contextlib.ExitStack()
        with ctx:
            cpool = ctx.enter_context(tc.tile_pool(name="consts", bufs=1))
            spool = ctx.enter_context(tc.tile_pool(name="work", bufs=3))
            gpool = ctx.enter_context(tc.tile_pool(name="gru", bufs=3))
            ppool_big = ctx.enter_context(
                tc.tile_pool(name="psbig", bufs=3, space="PSUM"))
            ppool_fix = ctx.enter_context(
                tc.tile_pool(name="psfix", bufs=1, space="PSUM"))

            # ---- load constants (3 DMAs, one per pack) ----
            cp_t = {}
            for p in CPACK_ROWS:
                cp_t[p] = cpool.tile([CPACK_ROWS[p], CPACK_COLS[p]],
                                     dt.float32, tag=f"cp{p}", name=f"cp{p}")
                nc.sync.dma_start(cp_t[p][:], cpk[p])

            def cslice(name, nrows):
                p, c0, w_ = CPACK_LAYOUT[name]
                return cp_t[p][0:nrows, c0:c0 + w_]

            st_s = cslice("st", 2)
            dy_s = cslice("dy", 2)
            swT_s = cslice("swT", 2)
            dwT_s = cslice("dwT", 2)
            wasT_s = cslice("wasT", H)
            wadT_s = cslice("wadT", H)
            wpsT_s = cslice("wpsT", H)
            wpcT_s = cslice("wpcT", H)
            wrT_s = cslice("wrT", H)
            whhT_s = cslice("whhT", H)
            whhn05T_s = cslice("whhn05T", H)
            ones64_s = cslice("ones64", S)
            vecs_s = cslice("vecs", H)
            biasrow_s = cslice("biasrow", 1)
            ones_s = cslice("ones_row", 1)
            wpc_s = cslice("wpc", H)
            stK_s = cslice("stK", 17)
            w2blk_s = cslice("w2blk", 17)

            # biasrow columns: [0:H]=static_b [H:2H]=dynamic_b
            #   [2H:5H]=Gtab gate biases (r,z incl bhh; n = gbias_n)
            #   [5H:8H]=gi0 rows (r,z incl bhh fold; n plain)
            #   [8H:9H]=0.5*bhh_n
            # vecs columns: 4=attn_v 5=ptr_v 6=iota64(rows 0:64)

            # ---- persistent state ----
            h_s = cpool.tile([H, BL], dt.float32, tag="h", name="h")
            nc.vector.memset(h_s[:], 0.0)
            h2_s = cpool.tile([H, BL], dt.float32, tag="h2", name="h2")
            nc.vector.memset(h2_s[:], 0.0)

            U_s = [cpool.tile([H, W], dt.float32, tag=f"U{g}", name=f"U{g}")
                   for g in range(NG)]
            V_s = [cpool.tile([H, W], dt.float32, tag=f"V{g}", name=f"V{g}")
                   for g in range(NG)]
            PST_s = [cpool.tile([S, GB * H], dt.float32, tag=f"PST{g}",
                                name=f"PST{g}") for g in range(NG)]
            GtT_s = [cpool.tile([S, 3 * GB * H], dt.float32, tag=f"GtT{g}",
                                name=f"GtT{g}") for g in range(NG)]
            Zbuf_s = [cpool.tile([1, S * GB], dt.float32, tag=f"Zb{g}",
                                 name=f"Zb{g}") for g in range(NG)]
            mxbuf_s = [cpool.tile([1, S * GB], dt.float32, tag=f"mxb{g}",
                                  name=f"mxb{g}") for g in range(NG)]
            oi_s = [cpool.tile([GB, S], dt.int32, tag=f"oi{g}", name=f"oi{g}")
                    for g in range(NG)]

            # persistent per-group psum scratch (one full bank each):
            #   pw [H,0:8] | qt [0:64,8:16] | w2p [H,16:24] | z [H,24:32]
            #   lt [0:64,32:40] | zr [0:1,40:48] | ic [0:8,48:49]
            fix = [ppool_fix.tile([H, 128], dt.float32, tag=f"fix{g}",
                                  name=f"fix{g}") for g in range(NG)]
            ghq_t = ppool_fix.tile([H, 128], dt.float32, tag="ghq",
                                   name="ghq")
            ghq_reg = [[ghq_t[:, (2 * g + e) * 32:(2 * g + e + 1) * 32]
                        for e in range(2)] for g in range(NG)]
            zbank = [ppool_fix.tile([H, 512], dt.float32, tag=f"zbk{g}",
                                    name=f"zbk{g}") for g in range(NG)]
            pw_r = [fx[:, 0:GB] for fx in fix]
            qt_r = [fx[0:S, GB:2 * GB] for fx in fix]
            w2p_r = [fx[:, 2 * GB:3 * GB] for fx in fix]
            z_r = [fx[:, 3 * GB:4 * GB] for fx in fix]
            lt_r = [fx[0:S, 4 * GB:5 * GB] for fx in fix]
            zr_r = [zb[0:1, :] for zb in zbank]
            ic_r = [zb[64:64 + GB, 0:S] for zb in zbank]

            # ---------------- precompute ----------------
            def colrange(g):
                return slice(g * W, (g + 1) * W)

            sh_s, dh_s = [], []
            for g in range(NG):
                cs = colrange(g)
                ps = ppool_big.tile([H, W], dt.float32, tag="pc", name="pc")
                nc.tensor.matmul(ps[:], swT_s[:], st_s[:, cs], start=True,
                                 stop=False)
                nc.tensor.matmul(ps[:], biasrow_s[:, 0:H], ones_s[:],
                                 start=False, stop=True)
                sh = cpool.tile([H, W], dt.float32, tag=f"sh{g}", name=f"sh{g}")
                nc.scalar.copy(sh[:], ps[:])
                sh_s.append(sh)
                pd = ppool_big.tile([H, W], dt.float32, tag="pc", name="pc")
                nc.tensor.matmul(pd[:], dwT_s[:], dy_s[:, cs], start=True,
                                 stop=False)
                nc.tensor.matmul(pd[:], biasrow_s[:, H:2 * H], ones_s[:],
                                 start=False, stop=True)
                dh = cpool.tile([H, W], dt.float32, tag=f"dh{g}", name=f"dh{g}")
                nc.vector.tensor_copy(dh[:], pd[:])
                dh_s.append(dh)

            def build_gtT(g):
                # GtT per gate via host-packed block-diagonal weights
                for k in range(3):
                    for hh in range(2):
                        pg = ppool_big.tile([S, W], dt.float32, tag="pc",
                                            name="gtT")
                        nc.tensor.matmul(
                            pg[:], stK_s[:, g * S:(g + 1) * S],
                            w2blk_s[:, k * GB * H + hh * W:
                                    k * GB * H + (hh + 1) * W],
                            start=True, stop=True)
                        dst = GtT_s[g][:, k * GB * H + hh * W:
                                       k * GB * H + (hh + 1) * W]
                        if hh == 0:
                            nc.scalar.copy(dst, pg[:])
                        else:
                            nc.vector.tensor_copy(dst, pg[:])

            for g in range(NG):
                cs = colrange(g)
                # U = W_as@sh + W_ad@dh
                pu = ppool_big.tile([H, W], dt.float32, tag="pc", name="pc")
                nc.tensor.matmul(pu[:], wasT_s[:], sh_s[g][:], start=True,
                                 stop=False)
                nc.tensor.matmul(pu[:], wadT_s[:], dh_s[g][:], start=False,
                                 stop=True)
                nc.scalar.copy(U_s[g][:], pu[:])
                # V = P_s@sh
                pv = ppool_big.tile([H, W], dt.float32, tag="pc", name="pc")
                nc.tensor.matmul(pv[:], wpsT_s[:], sh_s[g][:], start=True,
                                 stop=True)
                nc.vector.tensor_copy(V_s[g][:], pv[:])
                # PST_b = sh_b.T @ P_c.T via direct matmuls (no transposes)
                for b in range(GB):
                    pt = ppool_big.tile([S, H], dt.float32, tag="pc",
                                        name="pst_t")
                    nc.tensor.matmul(pt[:], sh_s[g][:, b * S:(b + 1) * S],
                                     wpcT_s[:], start=True, stop=True)
                    dstp = PST_s[g][:, b * H:(b + 1) * H]
                    if b % 2 == 0:
                        nc.scalar.copy(dstp, pt[:])
                    else:
                        nc.vector.tensor_copy(dstp, pt[:])
            build_gtT(0)

            # ---------------- decode loop ----------------
            gcols = [slice(g * GB, (g + 1) * GB) for g in range(NG)]
            psGHQ = [None, None]   # [H, 4*GB]: rz | NB | Q
            oh_t = [None, None]
            lTs_t = [None, None]
            mxr_t = [None, None]

            def gru_init(g):
                pg = ghq_reg[g][0]
                for k in range(2):
                    nc.tensor.matmul(pg[:, k * GB:(k + 1) * GB],
                                     biasrow_s[:, (5 + k) * H:(6 + k) * H],
                                     ones_s[:, 0:GB], start=True, stop=True,
                                     skip_group_check=True)
                nc.tensor.matmul(pg[:, 2 * GB:3 * GB],
                                 biasrow_s[:, 8 * H:9 * H],
                                 ones_s[:, 0:GB], start=True, stop=True,
                                 skip_group_check=True)
                nc.tensor.matmul(pg[:, 3 * GB:4 * GB],
                                 biasrow_s[:, 7 * H:8 * H],
                                 ones_s[:, 0:GB], start=True, stop=True,
                                 skip_group_check=True)
                psGHQ[g] = pg

            def gru(t, g):
                """psGHQ (whh@h + gi) -> gates -> h update; emits psW for
                this step's attention and whh part of psGHQ(t+1)."""
                cs = gcols[g]
                pg = psGHQ[g]
                th = gpool.tile([H, 2 * GB], dt.float32, tag="th", name="th")
                nc.scalar.activation(th[:], pg[:, 0:2 * GB], AF.Tanh,
                                     scale=0.5)
                t1 = gpool.tile([H, GB], dt.float32, tag="t1", name="t1")
                nc.vector.scalar_tensor_tensor(t1[:], th[:, 0:GB], 1.0,
                                               pg[:, 2 * GB:3 * GB],
                                               op0=ALU.add, op1=ALU.mult)
                na = gpool.tile([H, GB], dt.float32, tag="na", name="na")
                nc.vector.tensor_tensor(na[:], t1[:], pg[:, 3 * GB:4 * GB],
                                        op=ALU.add)
                # zh = z*h = (th_z+1)*(h/2); runs in the na/n shadow
                zh = gpool.tile([H, GB], dt.float32, tag="zh", name="zh")
                nc.vector.scalar_tensor_tensor(zh[:], th[:, GB:2 * GB], 1.0,
                                               h2_s[:, cs], op0=ALU.add,
                                               op1=ALU.mult)
                n_s = gpool.tile([H, GB], dt.float32, tag="n", name="n")
                nc.scalar.activation(n_s[:], na[:], AF.Tanh)
                # h' = (1-z)*n + z*h = -0.5*(th_z-1)*n + zh
                t_ = gpool.tile([H, GB], dt.float32, tag="t_", name="t_")
                nc.vector.scalar_tensor_tensor(t_[:], th[:, GB:2 * GB], 1.0,
                                               n_s[:], op0=ALU.subtract,
                                               op1=ALU.mult)
                nc.vector.scalar_tensor_tensor(h_s[:, cs], t_[:], -0.5,
                                               zh[:], op0=ALU.mult,
                                               op1=ALU.add)
                # psW = wrT@h'
                nc.tensor.matmul(pw_r[g], wrT_s[:], h_s[:, cs], start=True,
                                 stop=True, skip_group_check=True)
                nc.vector.tensor_scalar_mul(h2_s[:, cs], h_s[:, cs], 0.5)
                if t < n_steps - 1:
                    pg2 = ghq_reg[g][(t + 1) & 1]
                    for k in range(2):
                        nc.tensor.matmul(pg2[:, k * GB:(k + 1) * GB],
                                         whhT_s[:, k * H:(k + 1) * H],
                                         h_s[:, cs], start=True, stop=False,
                                         skip_group_check=True)
                    nc.tensor.matmul(pg2[:, 2 * GB:3 * GB], whhn05T_s[:],
                                     h_s[:, cs], start=True, stop=False,
                                     skip_group_check=True)
                    nc.tensor.matmul(pg2[:, 2 * GB:3 * GB],
                                     biasrow_s[:, 8 * H:9 * H],
                                     ones_s[:, 0:GB], start=False, stop=True,
                                     skip_group_check=True)
                    psGHQ[g] = pg2
                else:
                    psGHQ[g] = None

            def front(t, g):
                """Attention front: psA -> ea -> attn logits -> exp ->
                context matmuls."""
                pA = ppool_big.tile([H, W], dt.float32, tag="pc", name="pc")
                nc.vector.tensor_tensor(
                    pA[:].rearrange("p (b s) -> p b s", s=S),
                    U_s[g][:].rearrange("p (b s) -> p b s", s=S),
                    pw_r[g].unsqueeze(2).broadcast_to([H, GB, S]),
                    op=ALU.add)
                ea = spool.tile([H, W], dt.float32, tag="ea", name="ea")
                nc.scalar.activation(ea[:], pA[:], AF.Tanh)
                for b in range(GB):
                    nc.tensor.matmul(qt_r[g][:, b:b + 1],
                                     ea[:, b * S:(b + 1) * S],
                                     vecs_s[:, 4:5], start=True, stop=True,
                                     skip_group_check=True)
                qT = spool.tile([S, GB], dt.float32, tag="qT", name="qT")
                nc.scalar.activation(qT[:], qt_r[g], AF.Exp)
                for b in range(GB):
                    nc.tensor.matmul(w2p_r[g][:, b:b + 1],
                                     PST_s[g][:, b * H:(b + 1) * H],
                                     qT[:, b:b + 1], start=True, stop=True,
                                     skip_group_check=True)
                nc.tensor.matmul(z_r[g], ones64_s[:], qT[:], start=True,
                                 stop=True, skip_group_check=True)

            def back(t, g):
                """Attention back: softmax fold -> pointer tanh -> pointer
                logits -> partition max; qP/Z row for logp."""
                rz_s = gpool.tile([H, GB], dt.float32, tag="rz", name="rz")
                nc.vector.reciprocal(rz_s[:], z_r[g])
                w2 = gpool.tile([H, GB], dt.float32, tag="w2", name="w2")
                nc.vector.tensor_tensor(w2[:], w2p_r[g], rz_s[:], op=ALU.mult)
                pP = ppool_big.tile([H, W], dt.float32, tag="pc", name="pc")
                nc.vector.tensor_tensor(
                    pP[:].rearrange("p (b s) -> p b s", s=S),
                    V_s[g][:].rearrange("p (b s) -> p b s", s=S),
                    w2[:].unsqueeze(2).broadcast_to([H, GB, S]),
                    op=ALU.add)
                ep = spool.tile([H, W], dt.float32, tag="ep", name="ep")
                nc.scalar.activation(ep[:], pP[:], AF.Tanh)
                for b in range(GB):
                    nc.tensor.matmul(lt_r[g][:, b:b + 1],
                                     ep[:, b * S:(b + 1) * S],
                                     vecs_s[:, 5:6], start=True, stop=True,
                                     skip_group_check=True)
                qP = spool.tile([S, GB], dt.float32, tag="qP", name="qP")
                nc.scalar.activation(qP[:], lt_r[g], AF.Exp)
                mxr = spool.tile([S, GB], dt.float32, tag="mxr", name="mxr")
                nc.gpsimd.partition_all_reduce(
                    mxr[:], qP[:], channels=S,
                    reduce_op=bass_isa.ReduceOp.max)
                nc.tensor.matmul(zr_r[g][:, t * GB:(t + 1) * GB],
                                 ones64_s[:, 0:1], qP[:], start=True,
                                 stop=True, skip_group_check=True)
                lTs_t[g] = qP
                mxr_t[g] = mxr

            def tail(t, g):
                """Argmax one-hot; gi matmuls into psGHQ(t+1); oi/logp
                bookkeeping (all deps already satisfied here)."""
                oh = spool.tile([S, GB], dt.float32, tag="oh", name="oh")
                nc.vector.tensor_tensor(oh[:], lTs_t[g][:], mxr_t[g][:],
                                        op=ALU.is_equal)
                if t < n_steps - 1:
                    pg2 = psGHQ[g]
                    for k in range(2):
                        for b in range(GB):
                            nc.tensor.matmul(
                                pg2[:, k * GB + b:k * GB + b + 1],
                                GtT_s[g][:, (k * GB + b) * H:(k * GB + b + 1) * H],
                                oh[:, b:b + 1], start=False,
                                stop=(k == 1 and b == GB - 1),
                                skip_group_check=True)
                    for b in range(GB):
                        nc.tensor.matmul(
                            pg2[:, 3 * GB + b:3 * GB + b + 1],
                            GtT_s[g][:, (2 * GB + b) * H:(2 * GB + b + 1) * H],
                            oh[:, b:b + 1], start=(b == 0), stop=(b == GB - 1),
                            skip_group_check=True)
                nc.tensor.matmul(ic_r[g][:, t:t + 1], oh[:],
                                 vecs_s[0:S, 6:7], start=True, stop=True,
                                 skip_group_check=True)
                nc.gpsimd.tensor_copy(mxbuf_s[g][:, t * GB:(t + 1) * GB],
                                      mxr_t[g][0:1, :])

            # software-pipelined emission:
            #   body(t) = tailA(t-1) gruA(t) backB(t-1) frontA(t)
            #             tailB(t-1) gruB(t) backA(t) frontB(t)
            for g in range(NG):
                gru_init(g)
            for t in range(n_steps):
                if t > 0:
                    tail(t - 1, 0)
                gru(t, 0)
                if t > 0:
                    back(t - 1, 1)
                front(t, 0)
                if t > 0:
                    tail(t - 1, 1)
                gru(t, 1)
                if t == 0:
                    build_gtT(1)
                back(t, 0)
                front(t, 1)
            tail(n_steps - 1, 0)
            back(n_steps - 1, 1)
            tail(n_steps - 1, 1)

            # ---------------- epilogue ----------------
            ns = n_steps
            for g in range(NG):
                rz2 = spool.tile([1, S * GB], dt.float32, tag="rz2",
                                 name="rz2")
                nc.vector.reciprocal(rz2[:, 0:ns * GB],
                                     zr_r[g][:, 0:ns * GB])
                nc.vector.tensor_copy(oi_s[g][:, 0:ns], ic_r[g][:, 0:ns])
                rat = spool.tile([1, S * GB], dt.float32, tag="lnq",
                                 name="rat")
                nc.vector.tensor_tensor(rat[:, 0:ns * GB],
                                        mxbuf_s[g][:, 0:ns * GB],
                                        rz2[:, 0:ns * GB], op=ALU.mult)
                olp = spool.tile([1, S * GB], dt.float32, tag="olp",
                                 name="olp")
                nc.scalar.activation(olp[:, 0:ns * GB], rat[:, 0:ns * GB],
                                     AF.Ln)
                # olp free order is (t, b); DRAM wants [b, t]
                olp3 = olp[:, 0:ns * GB].rearrange("p (t b) -> p b t", b=GB)
                for b in range(GB):
                    nc.sync.dma_start(
                        out_logp[g * GB + b:g * GB + b + 1, 0:ns],
                        olp3[:, b, :])
                nc.sync.dma_start(out_idx[g * GB:(g + 1) * GB, 0:ns],
                                  oi_s[g][:, 0:ns])

    nc.compile()
    _legalize_waits(nc)
    return nc


def _legalize_waits(nc):
    """Engine instruction structs carry a limited number of sync waits
    (LDWEIGHTS: 1; ACT/DVE/Pool structs are similarly tight). Move extra
    waits onto injected same-engine nops placed immediately before."""
    import concourse.mybir as mybir

    CAPPED = {mybir.EngineType.PE, mybir.EngineType.Activation,
              mybir.EngineType.DVE, mybir.EngineType.Pool}
    blocks = []
    for f in nc.m.functions:
        for blk in f.blocks:
            blocks.append((blk, list(blk.instructions)))
    final = []
    for blk, insts in blocks:
        out = []
        for i in insts:
            si = i.sync_info
            if (i.engine in CAPPED and si is not None and si.on_wait
                    and len(si.on_wait) > 1
                    and type(i).__name__ != "InstNop"):
                for wt in si.on_wait[:-1]:
                    nop = nc.engines[i.engine].nop().ins
                    nop.sync_info = mybir.SyncInfo(on_wait=[wt], on_update=[])
                    out.append(nop)
                i.sync_info = mybir.SyncInfo(on_wait=[si.on_wait[-1]],
                                             on_update=si.on_update)
            out.append(i)
        final.append((blk, out))
    for blk, out in final:
        blk.instructions = out


def _host_prep(inputs):
    """Build per-core input maps (weight prepack + batch sharding)."""
    f32 = np.float32
    st = np.ascontiguousarray(inputs["static"], dtype=f32)    # [B,2,S]
    dy = np.ascontiguousarray(inputs["dynamic"], dtype=f32)
    x0 = np.asarray(inputs["x0"], dtype=f32)
    sw, sb = np.asarray(inputs["static_w"], f32), np.asarray(inputs["static_b"], f32)
    dw, db = np.asarray(inputs["dynamic_w"], f32), np.asarray(inputs["dynamic_b"], f32)
    decw, decb = np.asarray(inputs["decoder_w"], f32), np.asarray(inputs["decoder_b"], f32)
    wih, whh = np.asarray(inputs["gru_wih"], f32), np.asarray(inputs["gru_whh"], f32)
    bih, bhh = np.asarray(inputs["gru_bih"], f32), np.asarray(inputs["gru_bhh"], f32)
    av, aW = np.asarray(inputs["attn_v"], f32), np.asarray(inputs["attn_W"], f32)
    pv, pW = np.asarray(inputs["ptr_v"], f32), np.asarray(inputs["ptr_W"], f32)

    W2 = (wih @ decw).astype(f32)                  # [3H,2]
    gbias = (wih @ decb + bih).astype(f32)         # [3H]
    bias_r = (gbias[0:H] + bhh[0:H]).astype(f32)
    bias_z = (gbias[H:2 * H] + bhh[H:2 * H]).astype(f32)
    bias_n = gbias[2 * H:3 * H].astype(f32)
    bhh_n = bhh[2 * H:3 * H].astype(f32)
    gi0 = (W2 @ x0 + gbias).astype(f32)
    gi0 = gi0 + np.concatenate([bhh[0:2 * H], np.zeros(H, f32)])

    vecs = np.zeros((H, 8), f32)
    vecs[:, 4] = av
    vecs[:, 5] = pv
    vecs[0:S, 6] = np.arange(S, dtype=f32)

    biasrow = np.concatenate(
        [sb, db, bias_r, bias_z, bias_n, gi0, 0.5 * bhh_n]).reshape(1, 9 * H)

    # block-diagonal W2 for direct GtT construction: rows (b,c) [+ones],
    # cols (b',h): W2[kH+h, c] iff b == b'
    gate_bias = np.stack([bias_r, bias_z, bias_n], 0)      # [3,H]
    w2blk = np.zeros((17, 3 * GB * H), f32)
    for k in range(3):
        for b in range(GB):
            cols = slice((k * GB + b) * H, (k * GB + b + 1) * H)
            w2blk[2 * b:2 * b + 2, cols] = W2[k * H:(k + 1) * H, :].T
            w2blk[16, cols] = gate_bias[k]
    parts = {
        "swT": sw.T, "dwT": dw.T,
        "wasT": aW[:, 0:H].T, "wadT": aW[:, H:2 * H].T,
        "wpsT": pW[:, 0:H].T, "wpcT": pW[:, H:2 * H].T,
        "wrT": aW[:, 2 * H:3 * H].T,
        "whhT": np.concatenate([whh[k * H:(k + 1) * H, :].T for k in range(3)],
                               axis=1),
        "whhn05T": 0.5 * whh[2 * H:3 * H, :].T,
        "ones64": np.ones((S, H), f32),
        "vecs": vecs, "biasrow": biasrow,
        "ones_row": np.ones((1, W), f32),
        "wpc": pW[:, H:2 * H],
        "w2blk": w2blk,
    }
    packs = {p: np.zeros((CPACK_ROWS[p], CPACK_COLS[p]), f32)
             for p in CPACK_ROWS}
    for nme, arr in parts.items():
        p, c0, w_ = CPACK_LAYOUT[nme]
        arr = np.asarray(arr, f32)
        packs[p][0:arr.shape[0], c0:c0 + w_] = arr

    in_maps = []
    for c in range(NCORES):
        sl = slice(c * BL, (c + 1) * BL)
        pb = packs["b"].copy()
        _, c0, w_ = CPACK_LAYOUT["st"]
        pb[0:2, c0:c0 + w_] = st[sl].transpose(1, 0, 2).reshape(2, BL * S)
        _, c0, w_ = CPACK_LAYOUT["dy"]
        pb[0:2, c0:c0 + w_] = dy[sl].transpose(1, 0, 2).reshape(2, BL * S)
        pd_ = packs["d"].copy()
        _, c0, w_ = CPACK_LAYOUT["stK"]
        stc = st[sl]                                     # [BL,2,S]
        for g in range(NG):
            blk = stc[g * GB:(g + 1) * GB]               # [GB,2,S]
            pd_[0:16, c0 + g * S:c0 + (g + 1) * S] = blk.reshape(16, S)
            pd_[16, c0 + g * S:c0 + (g + 1) * S] = 1.0
        in_maps.append({"cpack_a": packs["a"], "cpack_b": pb,
                        "cpack_c": packs["c"], "cpack_d": pd_})
    return in_maps


def kernel(**inputs):
    _ensure_path()
    from concourse import bass_utils

    if "nc" not in _CACHE:
        _CACHE["nc"] = _build_program()
    nc = _CACHE["nc"]

    in_maps = _host_prep(inputs)
    res = bass_utils.run_bass_kernel_spmd(nc, in_maps, core_ids=list(range(NCORES)))
    ptrs = np.concatenate([r["out_idx"] for r in res.results], axis=0)
    logps = np.concatenate([r["out_logp"] for r in res.results], axis=0)
    return ptrs.astype(np.int32), logps.astype(np.float32)


# revision 27
# speedup vs baseline: 1.0424x; 1.0085x over previous
"""DRL4TSP pointer-network decode on 8 Trainium2 NeuronCores.

Data-parallel over batch (16 items/core, 2 software-pipelined groups of 8).
All parameters replicated; the 64-step greedy decode runs fully on-device.

Structure (per core, fp32 throughout):
  - Hoisted loop-invariants (computed on device by PE):
      U    = W_as@static_h + W_ad@dynamic_h      [H,(b,s)]
      V    = P_s@static_h                        [H,(b,s)]
      PST  = (P_c@static_h) transposed per item  [S,(b,H)]
      GtT  = ((gru_wih@decoder_w)@static + bias) transposed per
             (gate,item)                         [S,(gate,b,H)]
  - Per decode step, the serial chain is split into 4 phases
    (gru / attn-front / attn-back / argmax-tail) and the two groups are
    emitted software-pipelined so every engine's in-order stream always
    has ready work:
      argmax: pointer logits [S,(b)] psum -> gpsimd partition_all_reduce
      (max) -> DVE is_equal one-hot -> next gi via one-hot matmuls
      against GtT (bit-exact gather); ptr index via one-hot @ iota.
      logp = max - ln(sum exp(l)) banked per step, one Ln at the end.
"""

import numpy as np


def _ensure_path():
    import sys

    try:
        import concourse.bass  # noqa: F401
        return
    except ImportError:
        pass
    for p in ("/opt/trn_rl_repo", "/root/.axon_site/_ro/trn_rl_repo"):
        if p not in sys.path:
            sys.path.insert(0, p)
    import concourse.bass  # noqa: F401


B, S, H = 128, 64, 128
NCORES = 8
BL = B // NCORES          # 16 items per core
NG = 2                    # groups per core
GB = BL // NG             # 8 items per group
W = GB * S                # 512 free width per group
F32 = "float32"

# constant packs, split by row count to minimize DMA bytes:
#   pack "a": 128-row tensors; "b": 2-row; "c": 1-row
_CP_PACKS = {
    "a": [("wasT", H), ("wadT", H), ("wpsT", H), ("wpcT", H), ("wrT", H),
          ("whhT", 3 * H), ("whhn05T", H), ("ones64", H), ("vecs", 8),
          ("wpc", H)],
    "b": [("st", BL * S), ("dy", BL * S), ("swT", H), ("dwT", H)],
    "c": [("biasrow", 9 * H), ("ones_row", W)],
    "d": [("stK", NG * S), ("w2blk", 3 * GB * H)],
}
CPACK_ROWS = {"a": H, "b": 2, "c": 1, "d": 17}
CPACK_LAYOUT = {}
CPACK_COLS = {}
for _p, _lst in _CP_PACKS.items():
    _c = 0
    for _n, _w in _lst:
        CPACK_LAYOUT[_n] = (_p, _c, _w)
        _c += _w
    CPACK_COLS[_p] = _c

_CACHE: dict = {}


def _build_program(n_steps: int = S):
    _ensure_path()
    import concourse.bass as bass
    import concourse.bacc as bacc
    import concourse.mybir as mybir
    import concourse.bass_isa as bass_isa
    from concourse.tile import TileContext

    dt = mybir.dt
    AF = mybir.ActivationFunctionType
    ALU = mybir.AluOpType

    nc = bacc.Bacc("TRN2", target_bir_lowering=False, debug=False,
                   enable_asserts=False, num_devices=NCORES)

    # ---------------- DRAM I/O ----------------
    cpk = {p: nc.dram_tensor(f"cpack_{p}", [CPACK_ROWS[p], CPACK_COLS[p]],
                             dt.float32, kind="ExternalInput").ap()
           for p in CPACK_ROWS}
    out_idx = nc.dram_tensor("out_idx", [BL, S], dt.int32,
                             kind="ExternalOutput").ap()
    out_logp = nc.dram_tensor("out_logp", [BL, S], dt.float32,
                              kind="ExternalOutput").ap()

    with TileContext(nc) as tc:
        import contextlib

        ctx = contextlib.ExitStack()
        with ctx:
            cpool = ctx.enter_context(tc.tile_pool(name="consts", bufs=1))
            spool = ctx.enter_context(tc.tile_pool(name="work", bufs=3))
            gpool = ctx.enter_context(tc.tile_pool(name="gru", bufs=3))
            ppool_big = ctx.enter_context(
                tc.tile_pool(name="psbig", bufs=3, space="PSUM"))
            ppool_fix = ctx.enter_context(
                tc.tile_pool(name="psfix", bufs=1, space="PSUM"))

            # ---- load constants (3 DMAs, one per pack) ----
            cp_t = {}
            for p in CPACK_ROWS:
                cp_t[p] = cpool.tile([CPACK_ROWS[p], CPACK_COLS[p]],
                                     dt.float32, tag=f"cp{p}", name=f"cp{p}")
                nc.sync.dma_start(cp_t[p][:], cpk[p])

            def cslice(name, nrows):
                p, c0, w_ = CPACK_LAYOUT[name]
                return cp_t[p][0:nrows, c0:c0 + w_]

            st_s = cslice("st", 2)
            dy_s = cslice("dy", 2)
            swT_s = cslice("swT", 2)
            dwT_s = cslice("dwT", 2)
            wasT_s = cslice("wasT", H)
            wadT_s = cslice("wadT", H)
            wpsT_s = cslice("wpsT", H)
            wpcT_s = cslice("wpcT", H)
            wrT_s = cslice("wrT", H)
            whhT_s = cslice("whhT", H)
            whhn05T_s = cslice("whhn05T", H)
            ones64_s = cslice("ones64", S)
            vecs_s = cslice("vecs", H)
            biasrow_s = cslice("biasrow", 1)
            ones_s = cslice("ones_row", 1)
            wpc_s = cslice("wpc", H)
            stK_s = cslice("stK", 17)
            w2blk_s = cslice("w2blk", 17)

            # biasrow columns: [0:H]=static_b [H:2H]=dynamic_b
            #   [2H:5H]=Gtab gate biases (r,z incl bhh; n = gbias_n)
            #   [5H:8H]=gi0 rows (r,z incl bhh fold; n plain)
            #   [8H:9H]=0.5*bhh_n
            # vecs columns: 4=attn_v 5=ptr_v 6=iota64(rows 0:64)

            # ---- persistent state ----
            h_s = cpool.tile([H, BL], dt.float32, tag="h", name="h")
            nc.vector.memset(h_s[:], 0.0)
            h2_s = cpool.tile([H, BL], dt.float32, tag="h2", name="h2")
            nc.vector.memset(h2_s[:], 0.0)

            U_s = [cpool.tile([H, W], dt.float32, tag=f"U{g}", name=f"U{g}")
                   for g in range(NG)]
            V_s = [cpool.tile([H, W], dt.float32, tag=f"V{g}", name=f"V{g}")
                   for g in range(NG)]
            PST_s = [cpool.tile([S, GB * H], dt.float32, tag=f"PST{g}",
                                name=f"PST{g}") for g in range(NG)]
            GtT_s = [cpool.tile([S, 3 * GB * H], dt.float32, tag=f"GtT{g}",
                                name=f"GtT{g}") for g in range(NG)]
            Zbuf_s = [cpool.tile([1, S * GB], dt.float32, tag=f"Zb{g}",
                                 name=f"Zb{g}") for g in range(NG)]
            mxbuf_s = [cpool.tile([1, S * GB], dt.float32, tag=f"mxb{g}",
                                  name=f"mxb{g}") for g in range(NG)]
            oi_s = [cpool.tile([GB, S], dt.int32, tag=f"oi{g}", name=f"oi{g}")
                    for g in range(NG)]

            # persistent per-group psum scratch (one full bank each):
            #   pw [H,0:8] | qt [0:64,8:16] | w2p [H,16:24] | z [H,24:32]
            #   lt [0:64,32:40] | zr [0:1,40:48] | ic [0:8,48:49]
            fix = [ppool_fix.tile([H, 128], dt.float32, tag=f"fix{g}",
                                  name=f"fix{g}") for g in range(NG)]
            ghq_t = ppool_fix.tile([H, 128], dt.float32, tag="ghq",
                                   name="ghq")
            ghq_reg = [[ghq_t[:, (2 * g + e) * 32:(2 * g + e + 1) * 32]
                        for e in range(2)] for g in range(NG)]
            zbank = [ppool_fix.tile([H, 512], dt.float32, tag=f"zbk{g}",
                                    name=f"zbk{g}") for g in range(NG)]
            pw_r = [fx[:, 0:GB] for fx in fix]
            qt_r = [fx[0:S, GB:2 * GB] for fx in fix]
            w2p_r = [fx[:, 2 * GB:3 * GB] for fx in fix]
            z_r = [fx[:, 3 * GB:4 * GB] for fx in fix]
            lt_r = [fx[0:S, 4 * GB:5 * GB] for fx in fix]
            zr_r = [zb[0:1, :] for zb in zbank]
            ic_r = [zb[64:64 + GB, 0:S] for zb in zbank]

            # ---------------- precompute ----------------
            def colrange(g):
                return slice(g * W, (g + 1) * W)

            sh_s, dh_s = [], []
            for g in range(NG):
                cs = colrange(g)
                ps = ppool_big.tile([H, W], dt.float32, tag="pc", name="pc")
                nc.tensor.matmul(ps[:], swT_s[:], st_s[:, cs], start=True,
                                 stop=False)
                nc.tensor.matmul(ps[:], biasrow_s[:, 0:H], ones_s[:],
                                 start=False, stop=True)
                sh = cpool.tile([H, W], dt.float32, tag=f"sh{g}", name=f"sh{g}")
                nc.scalar.copy(sh[:], ps[:])
                sh_s.append(sh)
                pd = ppool_big.tile([H, W], dt.float32, tag="pc", name="pc")
                nc.tensor.matmul(pd[:], dwT_s[:], dy_s[:, cs], start=True,
                                 stop=False)
                nc.tensor.matmul(pd[:], biasrow_s[:, H:2 * H], ones_s[:],
                                 start=False, stop=True)
                dh = cpool.tile([H, W], dt.float32, tag=f"dh{g}", name=f"dh{g}")
                nc.vector.tensor_copy(dh[:], pd[:])
                dh_s.append(dh)

            def build_gtT(g):
                # GtT per gate via host-packed block-diagonal weights
                for k in range(3):
                    for hh in range(2):
                        pg = ppool_big.tile([S, W], dt.float32, tag="pc",
                                            name="gtT")
                        nc.tensor.matmul(
                            pg[:], stK_s[:, g * S:(g + 1) * S],
                            w2blk_s[:, k * GB * H + hh * W:
                                    k * GB * H + (hh + 1) * W],
                            start=True, stop=True)
                        dst = GtT_s[g][:, k * GB * H + hh * W:
                                       k * GB * H + (hh + 1) * W]
                        if hh == 0:
                            nc.scalar.copy(dst, pg[:])
                        else:
                            nc.vector.tensor_copy(dst, pg[:])

            for g in range(NG):
                cs = colrange(g)
                # U = W_as@sh + W_ad@dh
                pu = ppool_big.tile([H, W], dt.float32, tag="pc", name="pc")
                nc.tensor.matmul(pu[:], wasT_s[:], sh_s[g][:], start=True,
                                 stop=False)
                nc.tensor.matmul(pu[:], wadT_s[:], dh_s[g][:], start=False,
                                 stop=True)
                nc.scalar.copy(U_s[g][:], pu[:])
                # V = P_s@sh
                pv = ppool_big.tile([H, W], dt.float32, tag="pc", name="pc")
                nc.tensor.matmul(pv[:], wpsT_s[:], sh_s[g][:], start=True,
                                 stop=True)
                nc.vector.tensor_copy(V_s[g][:], pv[:])
                # PST_b = sh_b.T @ P_c.T via direct matmuls (no transposes)
                for b in range(GB):
                    pt = ppool_big.tile([S, H], dt.float32, tag="pc",
                                        name="pst_t")
                    nc.tensor.matmul(pt[:], sh_s[g][:, b * S:(b + 1) * S],
                                     wpcT_s[:], start=True, stop=True)
                    dstp = PST_s[g][:, b * H:(b + 1) * H]
                    if b % 2 == 0:
                        nc.scalar.copy(dstp, pt[:])
                    else:
                        nc.vector.tensor_copy(dstp, pt[:])
            build_gtT(0)

            # ---------------- decode loop ----------------
            gcols = [slice(g * GB, (g + 1) * GB) for g in range(NG)]
            psGHQ = [None, None]   # [H, 4*GB]: rz | NB | Q
            oh_t = [None, None]
            lTs_t = [None, None]
            mxr_t = [None, None]

            def gru_init(g):
                pg = ghq_reg[g][0]
                for k in range(2):
                    nc.tensor.matmul(pg[:, k * GB:(k + 1) * GB],
                                     biasrow_s[:, (5 + k) * H:(6 + k) * H],
                                     ones_s[:, 0:GB], start=True, stop=True,
                                     skip_group_check=True)
                nc.tensor.matmul(pg[:, 2 * GB:3 * GB],
                                 biasrow_s[:, 8 * H:9 * H],
                                 ones_s[:, 0:GB], start=True, stop=True,
                                 skip_group_check=True)
                nc.tensor.matmul(pg[:, 3 * GB:4 * GB],
                                 biasrow_s[:, 7 * H:8 * H],
                                 ones_s[:, 0:GB], start=True, stop=True,
                                 skip_group_check=True)
                psGHQ[g] = pg

            def gru(t, g):
                """psGHQ (whh@h + gi) -> gates -> h update; emits psW for
                this step's attention and whh part of psGHQ(t+1)."""
                cs = gcols[g]
                pg = psGHQ[g]
                th = gpool.tile([H, 2 * GB], dt.float32, tag="th", name="th")
                nc.scalar.activation(th[:], pg[:, 0:2 * GB], AF.Tanh,
                                     scale=0.5)
                t1 = gpool.tile([H, GB], dt.float32, tag="t1", name="t1")
                nc.vector.scalar_tensor_tensor(t1[:], th[:, 0:GB], 1.0,
                                               pg[:, 2 * GB:3 * GB],
                                               op0=ALU.add, op1=ALU.mult)
                na = gpool.tile([H, GB], dt.float32, tag="na", name="na")
                nc.vector.tensor_tensor(na[:], t1[:], pg[:, 3 * GB:4 * GB],
                                        op=ALU.add)
                # zh = z*h = (th_z+1)*(h/2); runs in the na/n shadow
                zh = gpool.tile([H, GB], dt.float32, tag="zh", name="zh")
                nc.vector.scalar_tensor_tensor(zh[:], th[:, GB:2 * GB], 1.0,
                                               h2_s[:, cs], op0=ALU.add,
                                               op1=ALU.mult)
                n_s = gpool.tile([H, GB], dt.float32, tag="n", name="n")
                nc.scalar.activation(n_s[:], na[:], AF.Tanh)
                # h' = (1-z)*n + z*h = -0.5*(th_z-1)*n + zh
                t_ = gpool.tile([H, GB], dt.float32, tag="t_", name="t_")
                nc.vector.scalar_tensor_tensor(t_[:], th[:, GB:2 * GB], 1.0,
                                               n_s[:], op0=ALU.subtract,
                                               op1=ALU.mult)
                nc.vector.scalar_tensor_tensor(h_s[:, cs], t_[:], -0.5,
                                               zh[:], op0=ALU.mult,
                                               op1=ALU.add)
                # psW = wrT@h'
                nc.tensor.matmul(pw_r[g], wrT_s[:], h_s[:, cs], start=True,
                                 stop=True, skip_group_check=True)
                nc.vector.tensor_scalar_mul(h2_s[:, cs], h_s[:, cs], 0.5)
                if t < n_steps - 1:
                    pg2 = ghq_reg[g][(t + 1) & 1]
                    for k in range(2):
                        nc.tensor.matmul(pg2[:, k * GB:(k + 1) * GB],
                                         whhT_s[:, k * H:(k + 1) * H],
                                         h_s[:, cs], start=True, stop=False,
                                         skip_group_check=True)
                    nc.tensor.matmul(pg2[:, 2 * GB:3 * GB], whhn05T_s[:],
                                     h_s[:, cs], start=True, stop=False,
                                     skip_group_check=True)
                    nc.tensor.matmul(pg2[:, 2 * GB:3 * GB],
                                     biasrow_s[:, 8 * H:9 * H],
                                     ones_s[:, 0:GB], start=False, stop=True,
                                     skip_group_check=True)
                    psGHQ[g] = pg2
                else:
                    psGHQ[g] = None

            def front(t, g):
                """Attention front: psA -> ea -> attn logits -> exp ->
                context matmuls."""
                pA = ppool_big.tile([H, W], dt.float32, tag="pc", name="pc")
                for hh in range(2):
                    sl = slice(hh * W // 2, (hh + 1) * W // 2)
                    nc.vector.tensor_tensor(
                        pA[:, sl].rearrange("p (b s) -> p b s", s=S),
                        U_s[g][:, sl].rearrange("p (b s) -> p b s", s=S),
                        pw_r[g][:, hh * GB // 2:(hh + 1) * GB // 2]
                        .unsqueeze(2).broadcast_to([H, GB // 2, S]),
                        op=ALU.add)
                ea = spool.tile([H, W], dt.float32, tag="ea", name="ea")
                nc.scalar.activation(ea[:], pA[:], AF.Tanh)
                for b in range(GB):
                    nc.tensor.matmul(qt_r[g][:, b:b + 1],
                                     ea[:, b * S:(b + 1) * S],
                                     vecs_s[:, 4:5], start=True, stop=True,
                                     skip_group_check=True)
                qT = spool.tile([S, GB], dt.float32, tag="qT", name="qT")
                nc.scalar.activation(qT[:], qt_r[g], AF.Exp)
                for b in range(GB):
                    nc.tensor.matmul(w2p_r[g][:, b:b + 1],
                                     PST_s[g][:, b * H:(b + 1) * H],
                                     qT[:, b:b + 1], start=True, stop=True,
                                     skip_group_check=True)
                nc.tensor.matmul(z_r[g], ones64_s[:], qT[:], start=True,
                                 stop=True, skip_group_check=True)

            def back(t, g):
                """Attention back: softmax fold -> pointer tanh -> pointer
                logits -> partition max; qP/Z row for logp."""
                rz_s = gpool.tile([H, GB], dt.float32, tag="rz", name="rz")
                nc.vector.reciprocal(rz_s[:], z_r[g])
                w2 = gpool.tile([H, GB], dt.float32, tag="w2", name="w2")
                nc.vector.tensor_tensor(w2[:], w2p_r[g], rz_s[:], op=ALU.mult)
                pP = ppool_big.tile([H, W], dt.float32, tag="pc", name="pc")
                for hh in range(2):
                    sl = slice(hh * W // 2, (hh + 1) * W // 2)
                    nc.vector.tensor_tensor(
                        pP[:, sl].rearrange("p (b s) -> p b s", s=S),
                        V_s[g][:, sl].rearrange("p (b s) -> p b s", s=S),
                        w2[:, hh * GB // 2:(hh + 1) * GB // 2]
                        .unsqueeze(2).broadcast_to([H, GB // 2, S]),
                        op=ALU.add)
                ep = spool.tile([H, W], dt.float32, tag="ep", name="ep")
                nc.scalar.activation(ep[:], pP[:], AF.Tanh)
                for b in range(GB):
                    nc.tensor.matmul(lt_r[g][:, b:b + 1],
                                     ep[:, b * S:(b + 1) * S],
                                     vecs_s[:, 5:6], start=True, stop=True,
                                     skip_group_check=True)
                qP = spool.tile([S, GB], dt.float32, tag="qP", name="qP")
                nc.scalar.activation(qP[:], lt_r[g], AF.Exp)
                mxr = spool.tile([S, GB], dt.float32, tag="mxr", name="mxr")
                nc.gpsimd.partition_all_reduce(
                    mxr[:], qP[:], channels=S,
                    reduce_op=bass_isa.ReduceOp.max)
                nc.tensor.matmul(zr_r[g][:, t * GB:(t + 1) * GB],
                                 ones64_s[:, 0:1], qP[:], start=True,
                                 stop=True, skip_group_check=True)
                lTs_t[g] = qP
                mxr_t[g] = mxr

            def tail(t, g):
                """Argmax one-hot; gi matmuls into psGHQ(t+1); oi/logp
                bookkeeping (all deps already satisfied here)."""
                oh = spool.tile([S, GB], dt.float32, tag="oh", name="oh")
                nc.vector.tensor_tensor(oh[:], lTs_t[g][:], mxr_t[g][:],
                                        op=ALU.is_equal)
                if t < n_steps - 1:
                    pg2 = psGHQ[g]
                    for k in range(2):
                        for b in range(GB):
                            nc.tensor.matmul(
                                pg2[:, k * GB + b:k * GB + b + 1],
                                GtT_s[g][:, (k * GB + b) * H:(k * GB + b + 1) * H],
                                oh[:, b:b + 1], start=False,
                                stop=(k == 1 and b == GB - 1),
                                skip_group_check=True)
                    for b in range(GB):
                        nc.tensor.matmul(
                            pg2[:, 3 * GB + b:3 * GB + b + 1],
                            GtT_s[g][:, (2 * GB + b) * H:(2 * GB + b + 1) * H],
                            oh[:, b:b + 1], start=(b == 0), stop=(b == GB - 1),
                            skip_group_check=True)
                nc.tensor.matmul(ic_r[g][:, t:t + 1], oh[:],
                                 vecs_s[0:S, 6:7], start=True, stop=True,
                                 skip_group_check=True)
                nc.gpsimd.tensor_copy(mxbuf_s[g][:, t * GB:(t + 1) * GB],
                                      mxr_t[g][0:1, :])

            # software-pipelined emission:
            #   body(t) = tailA(t-1) gruA(t) backB(t-1) frontA(t)
            #             tailB(t-1) gruB(t) backA(t) frontB(t)
            for g in range(NG):
                gru_init(g)
            for t in range(n_steps):
                if t > 0:
                    tail(t - 1, 0)
                gru(t, 0)
                if t > 0:
                    back(t - 1, 1)
                front(t, 0)
                if t > 0:
                    tail(t - 1, 1)
                gru(t, 1)
                if t == 0:
                    build_gtT(1)
                back(t, 0)
                front(t, 1)
            tail(n_steps - 1, 0)
            back(n_steps - 1, 1)
            tail(n_steps - 1, 1)

            # ---------------- epilogue ----------------
            ns = n_steps
            for g in range(NG):
                rz2 = spool.tile([1, S * GB], dt.float32, tag="rz2",
                                 name="rz2")
                nc.vector.reciprocal(rz2[:, 0:ns * GB],
                                     zr_r[g][:, 0:ns * GB])
                nc.vector.tensor_copy(oi_s[g][:, 0:ns], ic_r[g][:, 0:ns])
                rat = spool.tile([1, S * GB], dt.float32, tag="lnq",
                                 name="rat")
                nc.vector.tensor_tensor(rat[:, 0:ns * GB],
                                        mxbuf_s[g][:, 0:ns * GB],
                                        rz2[:, 0:ns * GB], op=ALU.mult)
                olp = spool.tile([1, S * GB], dt.float32, tag="olp",
                                 name="olp")
                nc.scalar.activation(olp[:, 0:ns * GB], rat[:, 0:ns * GB],
                                     AF.Ln)
                # olp free order is (t, b); DRAM wants [b, t]
                olp3 = olp[:, 0:ns * GB].rearrange("p (t b) -> p b t", b=GB)
                for b in range(GB):
                    nc.sync.dma_start(
                        out_logp[g * GB + b:g * GB + b + 1, 0:ns],
                        olp3[:, b, :])
                nc.sync.dma_start(out_idx[g * GB:(g + 1) * GB, 0:ns],
                                  oi_s[g][:, 0:ns])

    nc.compile()
    _legalize_waits(nc)
    return nc


def _legalize_waits(nc):
    """Engine instruction structs carry a limited number of sync waits
    (LDWEIGHTS: 1; ACT/DVE/Pool structs are similarly tight). Move extra
    waits onto injected same-engine nops placed immediately before."""
    import concourse.mybir as mybir

    CAPPED = {mybir.EngineType.PE, mybir.EngineType.Activation,
              mybir.EngineType.DVE, mybir.EngineType.Pool}
    blocks = []
    for f in nc.m.functions:
        for blk in f.blocks:
            blocks.append((blk, list(blk.instructions)))
    final = []
    for blk, insts in blocks:
        out = []
        for i in insts:
            si = i.sync_info
            if (i.engine in CAPPED and si is not None and si.on_wait
                    and len(si.on_wait) > 1
                    and type(i).__name__ != "InstNop"):
                for wt in si.on_wait[:-1]:
                    nop = nc.engines[i.engine].nop().ins
                    nop.sync_info = mybir.SyncInfo(on_wait=[wt], on_update=[])
                    out.append(nop)
                i.sync_info = mybir.SyncInfo(on_wait=[si.on_wait[-1]],
                                             on_update=si.on_update)
            out.append(i)
        final.append((blk, out))
    for blk, out in final:
        blk.instructions = out


def _host_prep(inputs):
    """Build per-core input maps (weight prepack + batch sharding)."""
    f32 = np.float32
    st = np.ascontiguousarray(inputs["static"], dtype=f32)    # [B,2,S]
    dy = np.ascontiguousarray(inputs["dynamic"], dtype=f32)
    x0 = np.asarray(inputs["x0"], dtype=f32)
    sw, sb = np.asarray(inputs["static_w"], f32), np.asarray(inputs["static_b"], f32)
    dw, db = np.asarray(inputs["dynamic_w"], f32), np.asarray(inputs["dynamic_b"], f32)
    decw, decb = np.asarray(inputs["decoder_w"], f32), np.asarray(inputs["decoder_b"], f32)
    wih, whh = np.asarray(inputs["gru_wih"], f32), np.asarray(inputs["gru_whh"], f32)
    bih, bhh = np.asarray(inputs["gru_bih"], f32), np.asarray(inputs["gru_bhh"], f32)
    av, aW = np.asarray(inputs["attn_v"], f32), np.asarray(inputs["attn_W"], f32)
    pv, pW = np.asarray(inputs["ptr_v"], f32), np.asarray(inputs["ptr_W"], f32)

    W2 = (wih @ decw).astype(f32)                  # [3H,2]
    gbias = (wih @ decb + bih).astype(f32)         # [3H]
    bias_r = (gbias[0:H] + bhh[0:H]).astype(f32)
    bias_z = (gbias[H:2 * H] + bhh[H:2 * H]).astype(f32)
    bias_n = gbias[2 * H:3 * H].astype(f32)
    bhh_n = bhh[2 * H:3 * H].astype(f32)
    gi0 = (W2 @ x0 + gbias).astype(f32)
    gi0 = gi0 + np.concatenate([bhh[0:2 * H], np.zeros(H, f32)])

    vecs = np.zeros((H, 8), f32)
    vecs[:, 4] = av
    vecs[:, 5] = pv
    vecs[0:S, 6] = np.arange(S, dtype=f32)

    biasrow = np.concatenate(
        [sb, db, bias_r, bias_z, bias_n, gi0, 0.5 * bhh_n]).reshape(1, 9 * H)

    # block-diagonal W2 for direct GtT construction: rows (b,c) [+ones],
    # cols (b',h): W2[kH+h, c] iff b == b'
    gate_bias = np.stack([bias_r, bias_z, bias_n], 0)      # [3,H]
    w2blk = np.zeros((17, 3 * GB * H), f32)
    for k in range(3):
        for b in range(GB):
            cols = slice((k * GB + b) * H, (k * GB + b + 1) * H)
            w2blk[2 * b:2 * b + 2, cols] = W2[k * H:(k + 1) * H, :].T
            w2blk[16, cols] = gate_bias[k]
    parts = {
        "swT": sw.T, "dwT": dw.T,
        "wasT": aW[:, 0:H].T, "wadT": aW[:, H:2 * H].T,
        "wpsT": pW[:, 0:H].T, "wpcT": pW[:, H:2 * H].T,
        "wrT": aW[:, 2 * H:3 * H].T,
        "whhT": np.concatenate([whh[k * H:(k + 1) * H, :].T for k in range(3)],
                               axis=1),
        "whhn05T": 0.5 * whh[2 * H:3 * H, :].T,
        "ones64": np.ones((S, H), f32),
        "vecs": vecs, "biasrow": biasrow,
        "ones_row": np.ones((1, W), f32),
        "wpc": pW[:, H:2 * H],
        "w2blk": w2blk,
    }
    packs = {p: np.zeros((CPACK_ROWS[p], CPACK_COLS[p]), f32)
             for p in CPACK_ROWS}
    for nme, arr in parts.items():
        p, c0, w_ = CPACK_LAYOUT[nme]
        arr = np.asarray(arr, f32)
        packs[p][0:arr.shape[0], c0:c0 + w_] = arr

    in_maps = []
    for c in range(NCORES):
        sl = slice(c * BL, (c + 1) * BL)
        pb = packs["b"].copy()
        _, c0, w_ = CPACK_LAYOUT["st"]
        pb[0:2, c0:c0 + w_] = st[sl].transpose(1, 0, 2).reshape(2, BL * S)
        _, c0, w_ = CPACK_LAYOUT["dy"]
        pb[0:2, c0:c0 + w_] = dy[sl].transpose(1, 0, 2).reshape(2, BL * S)
        pd_ = packs["d"].copy()
        _, c0, w_ = CPACK_LAYOUT["stK"]
        stc = st[sl]                                     # [BL,2,S]
        for g in range(NG):
            blk = stc[g * GB:(g + 1) * GB]               # [GB,2,S]
            pd_[0:16, c0 + g * S:c0 + (g + 1) * S] = blk.reshape(16, S)
            pd_[16, c0 + g * S:c0 + (g + 1) * S] = 1.0
        in_maps.append({"cpack_a": packs["a"], "cpack_b": pb,
                        "cpack_c": packs["c"], "cpack_d": pd_})
    return in_maps


def kernel(**inputs):
    _ensure_path()
    from concourse import bass_utils

    if "nc" not in _CACHE:
        _CACHE["nc"] = _build_program()
    nc = _CACHE["nc"]

    in_maps = _host_prep(inputs)
    res = bass_utils.run_bass_kernel_spmd(nc, in_maps, core_ids=list(range(NCORES)))
    ptrs = np.concatenate([r["out_idx"] for r in res.results], axis=0)
    logps = np.concatenate([r["out_logp"] for r in res.results], axis=0)
    return ptrs.astype(np.int32), logps.astype(np.float32)
